# revision 52
# baseline (speedup 1.0000x reference)
"""GNN (GENConv x2 + TopK pool) Bass/Tile kernel for TRN2, data-parallel over
8 NeuronCores (8 graphs per core).

Per-core layout conventions ("fm" = feature-major packed):
  - Edge tensors:  [128 = 16feat x 8graph, 16384 slots]  (dst-sorted per graph)
    row 16*g + f holds feature f of graph g; free axis = slot (graph-local).
  - Node tensors:  [128 = 16f x 8g, 512 nodes]  (e.g. h, hq, aggr)
    or [128 = 32f x 4g, 512] for 32-dim stages (halves A: graphs 0-3, B: 4-7).
  - Segment sums via chunked f32 prefix scan + boundary gather (ap_gather) in
    2 waves of 8192 slots to bound SBUF.
"""

import numpy as np
from contextlib import ExitStack

import concourse.bass as bass
import concourse.bacc as bacc
import concourse.mybir as mybir
import concourse.tile as tile
from concourse import library_config

F32 = mybir.dt.float32
F32R = mybir.dt.float32r
F16 = mybir.dt.float16
I16 = mybir.dt.int16
I8 = mybir.dt.int8

G = 8          # graphs per core
N = 512        # nodes per graph
EG = 16384     # edges per graph
EF = 16        # edge/node feature dim after encode
XF = 64        # input node feature dim
K = 256        # topk keep
SLOTS = 18432  # padded slots per graph (each node run padded to mult of 4)
Q4 = SLOTS // 4
HQ = Q4 // 2   # boundary-gather table split point
CHUNK = 1024   # slots per elementwise chunk
NCHUNK = SLOTS // CHUNK
PCH = 512      # slots per psum chunk (one bank)
NEND = 576     # padded end-list length (513 used)
EPS = 1e-7
BIGNEG = 2.0e9

USE_F32R = False   # set True once f32r numerics verified on HW
STOP_STAGE = None  # for HW bisects: "l1edge", "aggr1", "mlp1", "rank", "hq", "l2edge", "aggr2", "mlp2"


def mmdt(ap):
    return ap.bitcast(F32R) if USE_F32R else ap


# ----------------------------------------------------------------------------
# Host-side preprocessing: full inputs -> per-core named arrays
# ----------------------------------------------------------------------------

def prep_inputs(inputs: dict) -> list[dict]:
    x = np.asarray(inputs["x"], np.float32)            # [B*N, 64]
    ei = np.asarray(inputs["edge_index"])              # [2, E] int64
    ea = np.asarray(inputs["edge_attr"], np.float32)   # [E, 16]
    B = 64
    assert x.shape == (B * N, XF)
    assert ea.shape == (B * EG, EF)

    src_g = (ei[0] % N).astype(np.int64)
    dst_g = (ei[1] % N).astype(np.int64)
    graph_of_edge = (ei[0] // N).astype(np.int64)
    assert np.array_equal(graph_of_edge, np.repeat(np.arange(B), EG)), \
        "edge blocks not per-graph; prep assumes reference setup_inputs layout"
    assert np.array_equal(ei[0] // N, ei[1] // N)

    def lin(name):
        return np.asarray(inputs[name], np.float32)

    W_ne, b_ne = lin("W_ne"), lin("b_ne")
    W_ee, b_ee = lin("W_ee"), lin("b_ee")
    W1a, b1a, g1, be1 = lin("W1a"), lin("b1a"), lin("g1"), lin("be1")
    W1b, b1b = lin("W1b"), lin("b1b")
    W2a, b2a, g2, be2 = lin("W2a"), lin("b2a"), lin("g2"), lin("be2")
    W2b, b2b = lin("W2b"), lin("b2b")
    Wa, ba, Wo, bo = lin("Wa"), lin("ba"), lin("Wo"), lin("bo")
    w_pool = lin("w_pool")
    wp = w_pool / np.linalg.norm(w_pool)
    t1 = np.float32(inputs["t1"])
    t2 = np.float32(inputs["t2"])

    cst = {}
    wne = np.zeros((128, 4 * 128), np.float32)
    for p in range(4):
        for a in range(2):
            gg = 2 * p + a
            wne[64 * a:64 * a + XF, 128 * p + 16 * gg:128 * p + 16 * gg + EF] = W_ne
    cst["wne_stat"] = wne
    cst["bne_vec"] = np.tile(b_ne, G)[:, None].astype(np.float32)
    wee = np.zeros((128, 128), np.float32)
    for g in range(G):
        wee[16 * g:16 * g + EF, 16 * g:16 * g + EF] = W_ee
    cst["weeh_stat"] = wee.astype(np.float16)
    cst["bee_vec"] = np.tile(b_ee, G)[:, None].astype(np.float32)
    cst["ident"] = np.eye(128, dtype=np.float32)
    cst["identT"] = np.eye(128, dtype=np.float32)
    cst["identh"] = np.eye(128, dtype=np.float16)
    cst["t1vec"] = np.full((128, 1), t1, np.float32)
    cst["t2vec"] = np.full((128, 1), t2, np.float32)
    w1a = np.zeros((64, 128), np.float32)
    for gg in range(4):
        w1a[16 * gg:16 * gg + 16, 32 * gg:32 * gg + 32] = W1a
    cst["w1a_stat"] = np.vstack([w1a, w1a])
    cst["b1a_vec"] = np.tile(b1a, 4)[:, None].astype(np.float32)
    ones32 = np.zeros((128, 4), np.float32)
    for gg in range(4):
        ones32[32 * gg:32 * gg + 32, gg] = 1.0 / 32.0
    cst["ones32_stat"] = ones32
    onesb32 = np.zeros((4, 128), np.float32)
    for gg in range(4):
        onesb32[gg, 32 * gg:32 * gg + 32] = 1.0
    cst["onesb32_stat"] = onesb32
    cst["g1_vec"] = np.tile(g1, 4)[:, None].astype(np.float32)
    cst["be1_vec"] = np.tile(be1, 4)[:, None].astype(np.float32)
    w1b = np.zeros((64, 32), np.float32)
    for gg in range(2):
        w1b[32 * gg:32 * gg + 32, 16 * gg:16 * gg + 16] = W1b
    cst["w1b_stat"] = np.vstack([w1b, w1b])
    cst["b1b_vec"] = np.tile(b1b, G)[:, None].astype(np.float32)
    wpool = np.zeros((128, 8), np.float32)
    for g in range(G):
        wpool[16 * g:16 * g + EF, g] = wp
    cst["wpool_stat"] = wpool
    ones16b = np.zeros((8, 128), np.float32)
    for g in range(G):
        ones16b[g, 16 * g:16 * g + EF] = 1.0
    cst["ones16b_stat"] = ones16b
    cst["ones1x128"] = np.ones((1, 128), np.float32)
    cst["ones8x128"] = np.ones((8, 128), np.float32)
    onesel = np.zeros((8, 8 * 128), np.float32)
    for g in range(8):
        onesel[g, 128 * g:128 * (g + 1)] = 1.0
    cst["onesel_stat"] = onesel
    w2a = np.zeros((64, 128), np.float32)
    for gg in range(4):
        w2a[16 * gg:16 * gg + 16, 32 * gg:32 * gg + 32] = W2a
    cst["w2a_stat"] = np.vstack([w2a, w2a])
    cst["b2a_vec"] = np.tile(b2a, 4)[:, None].astype(np.float32)
    cst["g2_vec"] = np.tile(g2, 4)[:, None].astype(np.float32)
    cst["be2_vec"] = np.tile(be2, 4)[:, None].astype(np.float32)
    w2b = np.zeros((64, 64), np.float32)
    for gg in range(2):
        w2b[32 * gg:32 * gg + 32, 32 * gg:32 * gg + 32] = W2b
    cst["w2b_stat"] = np.vstack([w2b, w2b])
    cst["b2b_vec"] = np.tile(b2b, 4)[:, None].astype(np.float32)
    mbA = np.zeros((8, 128), np.float32)
    mbB = np.zeros((8, 128), np.float32)
    for g in range(4):
        mbA[g, 32 * g:32 * g + 32] = 1.0
        mbB[g + 4, 32 * g:32 * g + 32] = 1.0
    cst["maskbc_statA"] = mbA
    cst["maskbc_statB"] = mbB
    selk = np.zeros((128, 4 * 32), np.float32)
    for gg in range(4):
        selk[32 * gg:32 * gg + 32, 32 * gg:32 * gg + 32] = np.eye(32) / K
    cst["selk_stat"] = selk
    cst["wa_stat"] = Wa.astype(np.float32)
    cst["ba_vec"] = ba[:, None].astype(np.float32)
    cst["wo_stat"] = Wo.astype(np.float32)
    cst["bo2_vec"] = bo[:, None].astype(np.float32)
    cst["lneps_vec"] = np.full((4, 1), 1e-5, np.float32)
    cst["ones32h_stat"] = ones32.astype(np.float16)
    cst["w1bh_stat"] = cst["w1b_stat"].astype(np.float16)
    cst["w2bh_stat"] = cst["w2b_stat"].astype(np.float16)
    cst["big8_vec"] = np.full((128, 1), 1e8, np.float32)
    m1a = (w1a @ ones32).astype(np.float32)
    cst["m1a_stat"] = np.vstack([m1a, m1a])                      # [128, 4]
    cst["m1bias"] = (ones32.T @ np.tile(b1a, 4)[:, None]).astype(np.float32)
    m2a = (w2a @ ones32).astype(np.float32)
    cst["m2a_stat"] = np.vstack([m2a, m2a])
    cst["m2bias"] = (ones32.T @ np.tile(b2a, 4)[:, None]).astype(np.float32)

    core_maps = []
    for core in range(8):
        m = dict(cst)
        gsl = slice(core * G, (core + 1) * G)
        xt = np.zeros((128, 4 * 512), np.float32)
        xs = x.reshape(B, N, XF)[gsl]
        for p in range(4):
            for a in range(2):
                xt[64 * a:64 * a + XF, 512 * p:512 * (p + 1)] = xs[2 * p + a].T
        m["xT"] = xt

        attrT = np.zeros((128, SLOTS), np.float16)
        srcidx = np.full((128, SLOTS // 16), 512, np.int16)
        end4a = np.zeros((128, NEND // 16), np.int16)
        end4b = np.zeros((128, NEND // 16), np.int16)
        selm = np.zeros((128, NEND), np.int8)
        npad = np.zeros((128, N), np.float32)
        for gl in range(G):
            gid = core * G + gl
            s_l = src_g[gid * EG:(gid + 1) * EG]
            d_l = dst_g[gid * EG:(gid + 1) * EG]
            order = np.argsort(d_l, kind="stable")
            ds = d_l[order]
            ea_s = ea[gid * EG:(gid + 1) * EG][order]  # [EG, 16] dst-sorted
            ss = s_l[order]
            ends = np.searchsorted(ds, np.arange(N), side="right")
            starts = np.concatenate([[0], ends[:-1]])
            deg = ends - starts
            pdeg = (deg + 3) // 4 * 4
            off4 = np.concatenate([[0], np.cumsum(pdeg)])
            assert off4[-1] <= SLOTS
            # scatter real edges into padded positions
            pos = off4[ds] + (np.arange(EG) - starts[ds])
            at = np.zeros((SLOTS, EF), np.float16)
            at[pos] = ea_s.astype(np.float16)
            sp = np.full(SLOTS, 512, np.int16)
            sp[pos] = ss.astype(np.int16)
            attrT[16 * gl:16 * gl + EF, :] = at.T
            srcidx[16 * gl:16 * gl + 16, :] = sp.reshape(SLOTS // 16, 16).T
            e4 = np.zeros(NEND, np.int64)
            e4[:513] = off4 // 4
            e4[513:] = e4[512]
            ea4 = np.minimum(e4, HQ).astype(np.int16)
            eb4 = np.clip(e4 - HQ, 0, Q4 - HQ).astype(np.int16)
            end4a[16 * gl:16 * gl + 16, :] = ea4.reshape(NEND // 16, 16).T
            end4b[16 * gl:16 * gl + 16, :] = eb4.reshape(NEND // 16, 16).T
            selm[16 * gl:16 * gl + 16, :] = (e4 <= HQ).astype(np.int8)[None, :]
            npad[16 * gl:16 * gl + 16, :] = (pdeg - deg).astype(np.float32)[None, :]
        m["attrT"] = attrT
        m["srcidx"] = srcidx
        m["end4a"] = end4a
        m["end4b"] = end4b
        m["selm"] = selm
        m["npad"] = npad
        blob = np.zeros((128, CBLOB_BYTES), np.uint8)
        for name, shape, dt, off in CONST_SPECS:
            arr = m[name]
            bv = arr.view(np.uint8).reshape(arr.shape[0], -1)
            blob[:arr.shape[0], off:off + bv.shape[1]] = bv
        core_maps.append({"cblob": blob, "attrT": m["attrT"],
                          "ident": m["ident"],
                          "xT": m["xT"], "srcidx": m["srcidx"]})
    return core_maps


_CSPEC_RAW = [
    ("wne_stat", [128, 4 * 128], F32),
    ("bne_vec", [128, 1], F32),
    ("end4a", [128, NEND // 16], I16),
    ("end4b", [128, NEND // 16], I16),
    ("npad", [128, N], F32),
    ("weeh_stat", [128, 128], F16),
    ("identh", [128, 128], F16),
    ("bee_vec", [128, 1], F32),
    ("identT", [128, 128], F32),
    ("t1vec", [128, 1], F32),
    ("t2vec", [128, 1], F32),
    ("big8_vec", [128, 1], F32),
    ("w1a_stat", [128, 128], F32),
    ("b1a_vec", [128, 1], F32),
    ("ones32_stat", [128, 4], F32),
    ("onesb32_stat", [4, 128], F32),
    ("g1_vec", [128, 1], F32),
    ("be1_vec", [128, 1], F32),
    ("w1b_stat", [128, 32], F32),
    ("b1b_vec", [128, 1], F32),
    ("wpool_stat", [128, 8], F32),
    ("ones16b_stat", [8, 128], F32),
    ("ones1x128", [1, 128], F32),
    ("ones8x128", [8, 128], F32),
    ("onesel_stat", [8, 8 * 128], F32),
    ("w2a_stat", [128, 128], F32),
    ("b2a_vec", [128, 1], F32),
    ("g2_vec", [128, 1], F32),
    ("be2_vec", [128, 1], F32),
    ("w2b_stat", [128, 64], F32),
    ("b2b_vec", [128, 1], F32),
    ("maskbc_statA", [8, 128], F32),
    ("maskbc_statB", [8, 128], F32),
    ("selk_stat", [128, 4 * 32], F32),
    ("wa_stat", [32, 128], F32),
    ("ba_vec", [128, 1], F32),
    ("wo_stat", [128, 4], F32),
    ("bo2_vec", [4, 1], F32),
    ("lneps_vec", [4, 1], F32),
    ("ones32h_stat", [128, 4], F16),
    ("w1bh_stat", [128, 32], F16),
    ("w2bh_stat", [128, 64], F16),
    ("selm", [128, NEND], I8),
    ("m1a_stat", [128, 4], F32),
    ("m1bias", [4, 1], F32),
    ("m2a_stat", [128, 4], F32),
    ("m2bias", [4, 1], F32),
]

def _mk_const_specs():
    specs = []
    off = 0
    import numpy as _np
    for nm, shape, dt in _CSPEC_RAW:
        nbytes = int(_np.prod(shape[1:])) * mybir.dt.size(dt) if len(shape) > 1 else mybir.dt.size(dt)
        nbytes = int(_np.prod(shape[1:])) * mybir.dt.size(dt)
        specs.append((nm, shape, dt, off))
        off += (nbytes + 127) // 128 * 128
    return specs, off

CONST_SPECS, CBLOB_BYTES = _mk_const_specs()

INPUT_SPECS = [
    ("cblob", [128, None], mybir.dt.uint8),
    ("attrT", [128, SLOTS], F16),
    ("ident", [128, 128], F32R),
    ("xT", [128, 4 * 512], F32),
    ("srcidx", [128, SLOTS // 16], I16),
]
INPUT_SPECS[0] = ("cblob", [128, CBLOB_BYTES], mybir.dt.uint8)


# ----------------------------------------------------------------------------
# Device graph
# ----------------------------------------------------------------------------

def build_nc(debug_keys=(), n_rep=1):
    nc = bacc.Bacc(None, target_bir_lowering=False, debug=False)
    A = {}
    for name, shape, dt in INPUT_SPECS:
        A[name] = nc.declare_dram_parameter(name, shape, dt, isOutput=False)[:]
    out_ext = nc.declare_dram_parameter("out", [4, G], F32, isOutput=True)[:]
    dbg_ext = {}
    dbg_shapes = {
        "hT": ((128, N), F32), "hsrc0": ((128, CHUNK), F32),
        "m0": ((128, CHUNK), F16), "ev0": ((128, CHUNK), F16),
        "S0w0": ((128, Q4 + 1), F32), "S1w0": ((128, Q4 + 1), F32),
        "G0": ((128, NEND), F32), "G1": ((128, NEND), F32),
        "aggr1": ((128, N), F32), "h1": ((128, N), F32),
        "score": ((8, N), F32), "snode": ((128, 32), F32),
        "rk": ((128, 32), F32), "mask": ((128, 32), F32),
        "hq": ((128, N), F32), "hp": ((128, N), F32),
        "aggr2": ((128, N), F32), "h2a": ((128, N), F16),
        "h2b": ((128, N), F16), "pooled": ((32, 8), F32),
        "u1": ((128, N), F32), "y1sA": ((128, N), F32), "r1A": ((128, N), F32),
    }
    for key in debug_keys:
        shape, dt = dbg_shapes[key]
        dbg_ext[key] = nc.declare_dram_parameter(
            "dbg_" + key, list(shape), dt, isOutput=True)[:]

    with tile.TileContext(nc) as tc, ExitStack() as ctx:
        consts = ctx.enter_context(tc.tile_pool(name="consts", bufs=1))
        nodep = ctx.enter_context(tc.tile_pool(name="nodep", bufs=1))
        scanp = ctx.enter_context(tc.tile_pool(name="scanp", bufs=1))
        work = ctx.enter_context(tc.tile_pool(name="work", bufs=2))
        psum1 = ctx.enter_context(tc.tile_pool(name="psum1", bufs=2, space="PSUM"))
        psume = ctx.enter_context(tc.tile_pool(name="psume", bufs=2, space="PSUM"))
        psumq = ctx.enter_context(tc.tile_pool(name="psumq", bufs=2, space="PSUM"))
        psum = psume
        psumw = psume

        nc.gpsimd.load_library(library_config.ap_gather)

        C = {}
        cb = consts.tile([128, CBLOB_BYTES], mybir.dt.uint8, name="cblob_sb")
        _split = CONST_SPECS[12][3]
        nc.sync.dma_start(out=cb[:, :_split], in_=A["cblob"][:, :_split])
        for _n, _shape, _dt in (("srcidx", [128, SLOTS // 16], I16),
                                ("ident", [128, 128], F32R)):
            _t = consts.tile(_shape, _dt, name="c_" + _n)
            nc.scalar.dma_start(out=_t, in_=A[_n])
            C[_n] = _t
        _rest = CBLOB_BYTES - _split
        _step = (_rest + 3) // 4
        for _k in range(4):
            _a = _split + _k * _step
            _b = min(_split + (_k + 1) * _step, CBLOB_BYTES)
            if _a < _b:
                nc.scalar.dma_start(out=cb[:, _a:_b], in_=A["cblob"][:, _a:_b])
        for name, shape, dt, off in CONST_SPECS:
            nbytes = int(np.prod(shape[1:])) * mybir.dt.size(dt)
            ap = cb[:shape[0], off:off + nbytes].bitcast(dt)
            if len(shape) == 3:
                ap = ap.rearrange("p (a b) -> p a b", b=shape[2])
            C[name] = ap

        dbg_keys_set = set(debug_keys)

        def dbg(key, ap):
            if key in dbg_keys_set:
                nc.sync.dma_start(out=dbg_ext[key], in_=ap)

        for rep in range(n_rep):
            run_once(nc, tc, A, C, out_ext, dbg, nodep, scanp, work,
                     psum, psum1, psume, psumq, psumw, rep)
    nc.compile()
    return nc


def run_once(nc, tc, A, C, out_ext, dbg, nodep, scanp, work, psum, psum1,
             psume, psumq, psumw, rep):
    V = nc.vector
    S = nc.scalar
    T = nc.tensor
    Alu = mybir.AluOpType
    Act = mybir.ActivationFunctionType

    # ================= node encode: hT = x @ W_ne + b_ne =================
    hraw = psum.tile([128, N], F32, name="hraw", tag="zc")
    for half in range(2):
        xts = work.tile([128, 1024], F32, name=f"xts{half}", tag="xts")
        nc.sync.dma_start(out=xts, in_=A["xT"][:, 1024 * half:1024 * (half + 1)])
        for pp in range(2):
            p = 2 * half + pp
            T.matmul(hraw, mmdt(C["wne_stat"][:, 128 * p:128 * (p + 1)]),
                     mmdt(xts[:, 512 * pp:512 * (pp + 1)]),
                     start=(p == 0), stop=(p == 3))
    hT = nodep.tile([128, N + 1, 1], F32, name="hT")
    S.activation(hT[:, 0:N, 0], hraw, Act.Identity, bias=C["bne_vec"], scale=1.0)
    V.memset(hT[:, N:N + 1, 0], -BIGNEG)  # sentinel column for pad slots
    dbg("hT", hT[:, 0:N, 0])
    zeros512 = nodep.tile([128, PCH], F32, name="zeros512")
    V.memset(zeros512, 0.0)

    # ================= shared edge-phase machinery =================
    # Edge slots are dst-sorted with每 node's run padded to a multiple of 4
    # (pad slots gather the sentinel column -> mc=0, ev=1 in L1 / 0 in L2).
    # Per chunk: zc accumulates in PSUM (wee@attr + ident@hsrc), then 4:1
    # block sums via strided identity matmuls -> short scan -> small gather.
    def edge_layer(layer, table, tvec, dbg_pref):
        S4 = [scanp.tile([128, Q4 + 1, 1], F32, name=f"S4{ti}_{layer}_{rep}",
                         tag=f"S4{ti}") for ti in range(2)]
        V.memset(S4[0][:, 0:1, 0], 0.0)
        V.memset(S4[1][:, 0:1, 0], 0.0)

        for cc in range(NCHUNK):
            base = cc * CHUNK
            attrc = work.tile([128, CHUNK], F16, name="attrc", tag="attrc", bufs=3)
            nc.sync.dma_start(out=attrc, in_=A["attrT"][:, base:base + CHUNK])
            hsrc = work.tile([128, CHUNK, 1], F32, name="hsrc", tag="hsrc", bufs=3)
            j0 = base // 16
            nc.gpsimd.ap_gather(
                hsrc, table, C["srcidx"][:, j0:j0 + CHUNK // 16],
                channels=128, num_elems=N + 1, d=1, num_idxs=CHUNK)
            eap = psume.tile([128, CHUNK], F32, name="zc", tag="zc")
            for s in range(CHUNK // PCH):
                T.matmul(eap[:, PCH * s:PCH * (s + 1)], C["weeh_stat"],
                         attrc[:, PCH * s:PCH * (s + 1)],
                         start=True, stop=True)
            zc = work.tile([128, CHUNK], F32, name="zcs", tag="zcs", bufs=3)
            V.tensor_tensor(out=zc, in0=hsrc[:, :, 0], in1=eap, op=Alu.add)
            mc = work.tile([128, CHUNK], F16, name="mc", tag="mc", bufs=3)
            S.activation(mc, zc, Act.Relu, bias=C["bee_vec"], scale=1.0)
            if layer == 0 and base == 0:
                dbg("hsrc0", hsrc[:, :, 0])
            # evc | evmc packed side by side so the 4:1 reduce runs as one
            # 4-matmul group over both arrays.
            evp = work.tile([128, 2 * CHUNK], F16, name="evp", tag="evp",
                            bufs=3)
            evc = evp[:, 0:CHUNK]
            if layer == 0:
                S.activation(evc, mc, Act.Exp, bias=0.0, scale=tvec)
            else:
                sgn = work.tile([128, CHUNK], F16, name="sgn", tag="sgn")
                S.activation(sgn, zc, Act.Sign, bias=C["big8_vec"], scale=1.0)
                ev0 = work.tile([128, CHUNK], F16, name="ev0t", tag="ev0t",
                                bufs=2)
                S.activation(ev0, mc, Act.Exp, bias=0.0, scale=tvec)
                V.scalar_tensor_tensor(out=evc, in0=sgn, scalar=0.0, in1=ev0,
                                       op0=Alu.max, op1=Alu.mult)
            V.tensor_tensor(out=evp[:, CHUNK:2 * CHUNK], in0=evc, in1=mc,
                            op=Alu.mult)
            if layer == 0 and base == 0:
                dbg("m0", mc); dbg("ev0", evc)
            lo = cc * (CHUNK // 4)
            q4 = psumq.tile([128, CHUNK // 2], F32, name="q4", tag="q4")
            a3 = evp.rearrange("p (a b) -> p a b", b=4)
            for j in range(4):
                T.matmul(q4, C["identh"], a3[:, :, j],
                         start=(j == 0), stop=(j == 3))
            for ti in range(2):
                V.tensor_tensor_scan(
                    out=S4[ti][:, lo + 1:lo + 1 + CHUNK // 4, 0],
                    data0=q4[:, ti * (CHUNK // 4):(ti + 1) * (CHUNK // 4)],
                    data1=zeros512[:, 0:CHUNK // 4],
                    initial=S4[ti][:, lo:lo + 1, 0],
                    op0=Alu.add, op1=Alu.add)

        D = []
        GA = []
        for ti in range(2):
            # first-half gather only needs chunks 0..8 -> fires mid-layer
            gwa = nodep.tile([128, NEND, 1], F32,
                             name=f"gwa{ti}_{layer}_{rep}", tag=f"gwa{ti}")
            nc.gpsimd.ap_gather(
                gwa, S4[ti][:, 0:HQ + 1, :], C["end4a"],
                channels=128, num_elems=HQ + 1, d=1, num_idxs=NEND)
            GA.append(gwa)
        for ti in range(2):
            gwb = nodep.tile([128, NEND, 1], F32,
                             name=f"gwb{ti}_{layer}_{rep}", tag=f"gwb{ti}")
            nc.gpsimd.ap_gather(
                gwb, S4[ti][:, HQ:Q4 + 1, :], C["end4b"],
                channels=128, num_elems=Q4 - HQ + 1, d=1, num_idxs=NEND)
            gsel = nodep.tile([128, NEND], F32,
                              name=f"gsel{ti}_{layer}_{rep}", tag=f"gsel{ti}")
            V.select(gsel, C["selm"], GA[ti][:, :, 0], gwb[:, :, 0])
            dbg("G0" if ti == 0 else "G1", gsel)
            d = nodep.tile([128, N], F32, name=f"D{ti}_{layer}_{rep}",
                           tag=f"D{ti}")
            V.tensor_tensor(out=d, in0=gsel[:, 1:N + 1], in1=gsel[:, 0:N],
                            op=Alu.subtract)
            D.append(d)
        if layer == 0:
            d0c = nodep.tile([128, N], F32, name=f"D0c_{layer}_{rep}",
                             tag="D0c")
            V.tensor_tensor(out=d0c, in0=D[0], in1=C["npad"], op=Alu.subtract)
            D[0] = d0c
        return D

    def aggr_from_D(D0, D1, layer):
        dm = nodep.tile([128, N], F32, name=f"dm_{layer}_{rep}", tag="dm")
        V.tensor_scalar(dm, D0, 1e-16, None, Alu.max)
        rec = nodep.tile([128, N], F32, name=f"rec_{layer}_{rep}", tag="rec")
        V.reciprocal(rec, dm)
        ag = nodep.tile([128, N], F32, name=f"ag_{layer}_{rep}", tag="ag")
        V.tensor_tensor(out=ag, in0=D1, in1=rec, op=Alu.mult)
        return ag

    def mlp(uin, wa_stat, ba_vec, gvec, bevec, wb_stat, layer):
        N2 = 2 * N
        y1p = psumw.tile([128, 2, N], F32, name=f"y1p_{layer}", tag="zc")
        for half in range(2):
            T.matmul(y1p[:, half, :], mmdt(wa_stat[64 * half:64 * half + 64, :]),
                     mmdt(uin[64 * half:64 * half + 64, :]),
                     start=True, stop=True)
        y1pf = y1p.rearrange("p a b -> p (a b)")
        y1s = nodep.tile([128, N2], F32, name=f"y1s_{layer}_{rep}", tag="y1s")
        S.activation(y1s, y1pf, Act.Identity, bias=ba_vec, scale=1.0)
        if layer == 0:
            dbg("y1sA", y1s[:, 0:N])
        mstat = C["m1a_stat"] if layer == 0 else C["m2a_stat"]
        mbias = C["m1bias"] if layer == 0 else C["m2bias"]
        mp = psumw.tile([4, 2, N], F32, name=f"mp_{layer}", tag="zc")
        for half in range(2):
            T.matmul(mp[:, half, :], mmdt(mstat[64 * half:64 * half + 64, :]),
                     mmdt(uin[64 * half:64 * half + 64, :]),
                     start=True, stop=True)
        ms = nodep.tile([4, N2], F32, name=f"ms_{layer}_{rep}", tag="st4", bufs=2)
        S.activation(ms, mp.rearrange("p a b -> p (a b)"), Act.Identity,
                     bias=mbias, scale=1.0)
        mb = psumw.tile([128, 2, N], F32, name=f"mb_{layer}", tag="zc")
        for half in range(2):
            T.matmul(mb[:, half, :], mmdt(C["onesb32_stat"]),
                     mmdt(ms[:, half * N:half * N + N]), start=True, stop=True)
        yc = nodep.tile([128, N2], F32, name=f"yc_{layer}_{rep}", tag="yc")
        V.tensor_tensor(out=yc, in0=y1s, in1=mb.rearrange("p a b -> p (a b)"),
                        op=Alu.subtract)
        sq = nodep.tile([128, N2], F16, name=f"sq_{layer}_{rep}", tag="sq")
        S.activation(sq, yc, Act.Square, bias=0.0, scale=1.0)
        vp = psumw.tile([4, 2, N], F32, name=f"vp_{layer}", tag="zc")
        for half in range(2):
            T.matmul(vp[:, half, :], C["ones32h_stat"],
                     sq[:, half * N:half * N + N], start=True, stop=True)
        lnv = nodep.tile([4, N2], F32, name=f"lnv_{layer}_{rep}", tag="st4", bufs=2)
        S.activation(lnv, vp.rearrange("p a b -> p (a b)"), Act.Ln,
                     bias=C["lneps_vec"], scale=1.0)
        rstd = nodep.tile([4, N2], F32, name=f"rstd_{layer}_{rep}", tag="st4", bufs=2)
        S.activation(rstd, lnv, Act.Exp, bias=0.0, scale=-0.5)
        rb = psumw.tile([128, 2, N], F32, name=f"rb_{layer}", tag="zc")
        for half in range(2):
            T.matmul(rb[:, half, :], mmdt(C["onesb32_stat"]),
                     mmdt(rstd[:, half * N:half * N + N]), start=True, stop=True)
        vnorm = nodep.tile([128, N2], F32, name=f"vn_{layer}_{rep}", tag="vn")
        V.tensor_tensor(out=vnorm, in0=yc, in1=rb.rearrange("p a b -> p (a b)"),
                        op=Alu.mult)
        r1 = nodep.tile([128, N2], F16, name=f"r1_{layer}_{rep}", tag="r1")
        S.activation(r1, vnorm, Act.Relu, bias=bevec, scale=gvec)
        if layer == 0:
            dbg("r1A", r1[:, 0:N])
        M = wb_stat.shape[1]
        outs = [psum.tile([2 * M, N], F32, name=f"yqb{b}_{layer}", tag="zc")
                for b in range(2)]
        idx = 0
        for half in range(2):
            for q in range(2):
                bank, slot = divmod(idx, 2)
                T.matmul(outs[bank][slot * M:(slot + 1) * M, :],
                         wb_stat[64 * q:64 * q + 64, :],
                         r1[64 * q:64 * q + 64, half * N:half * N + N],
                         start=True, stop=True, skip_group_check=True)
                idx += 1
        return outs

    # ================= Layer 1 =================
    D0, D1 = edge_layer(0, hT, C["t1vec"], "")
    if STOP_STAGE == "l1edge":
        return stop_dma(D0[0:1, 0:1])
    aggr1 = aggr_from_D(D0, D1, 0)
    dbg("aggr1", aggr1)
    if STOP_STAGE == "aggr1":
        return stop_dma(aggr1[0:1, 0:1])
    u1 = nodep.tile([128, N], F32, name=f"u1_{rep}", tag="u1")
    V.scalar_tensor_tensor(out=u1, in0=aggr1, scalar=EPS, in1=hT[:, 0:N, 0],
                           op0=Alu.add, op1=Alu.add)
    dbg("u1", u1)
    y2q = mlp(u1, C["w1a_stat"], C["b1a_vec"], C["g1_vec"], C["be1_vec"],
              C["w1bh_stat"], 0)
    h1 = nodep.tile([128, N], F32, name=f"h1_{rep}", tag="h1")
    for b in range(2):
        S.activation(h1[64 * b:64 * (b + 1), :], y2q[b], Act.Relu,
                     bias=C["b1b_vec"][64 * b:64 * (b + 1), :], scale=1.0)
    dbg("h1", h1)
    if STOP_STAGE == "mlp1":
        return stop_dma(h1[0:1, 0:1])

    # ================= score / topk mask / gates =================
    scp = psum.tile([8, N], F32, name="scp", tag="zc")
    T.matmul(scp, C["wpool_stat"], h1, start=True, stop=True)
    scs = nodep.tile([8, N], F32, name=f"scs_{rep}", tag="scs")
    S.activation(scs, scp, Act.Copy, bias=0.0, scale=1.0)
    dbg("score", scs)
    snode = nodep.tile([128, 4, 8], F32, name=f"snode_{rep}", tag="snode")
    for t in range(4):
        tp = psum1.tile([128, 8], F32, name="tp", tag="small")
        T.transpose(tp, scs[:, 128 * t:128 * (t + 1)], C["identT"][0:8, 0:8])
        S.activation(snode[:, t, :], tp, Act.Copy, bias=0.0, scale=1.0)
    dbg("snode", snode.rearrange("p a b -> p (a b)"))
    sneg = nodep.tile([128, 4, 8], F32, name=f"sneg_{rep}", tag="sneg")
    V.tensor_scalar(sneg, snode, -1.0, None, Alu.mult)
    rk = nodep.tile([128, 4, 8], F32, name=f"rk_{rep}", tag="rk")
    sgnscratch = work.tile([128, N], F16, name="sgnscratch", tag="plscratch", bufs=1)
    gtscratch = work.tile([128, N], F16, name="gtscratch", tag="gtscratch", bufs=1)
    for g in range(G):
        sb = psum.tile([128, N], F32, name="sb", tag="zc")
        T.matmul(sb, mmdt(C["onesel_stat"][:, 128 * g:128 * (g + 1)]),
                 mmdt(scs), start=True, stop=True)
        # t 0/1 on Act (rk = sum of signs, keep iff rk <= -1);
        # t 2/3 on DVE (rk = count greater, keep iff rk <= 255.5)
        for t in range(2):
            S.activation(sgnscratch, sb, Act.Sign,
                         bias=sneg[:, t, g:g + 1], scale=1.0,
                         accum_out=rk[:, t, g:g + 1])
        for t in range(2, 4):
            V.tensor_scalar(gtscratch, sb, snode[:, t, g:g + 1], 0.0,
                            Alu.is_gt, Alu.add,
                            accum_out=rk[:, t, g:g + 1])
    dbg("rk", rk.rearrange("p a b -> p (a b)"))
    tst3 = nodep.tile([128, 4, 96], F32, name=f"tst3_{rep}", tag="tst3")
    mask01 = tst3[:, :, 64:72]
    V.tensor_scalar(mask01[:, 0:2, :], rk[:, 0:2, :], -1.0, None, Alu.is_le)
    V.tensor_scalar(mask01[:, 2:4, :], rk[:, 2:4, :], 255.5, None, Alu.is_le)
    th = nodep.tile([128, 4, 8], F32, name=f"th_{rep}", tag="th")
    S.activation(th, snode, Act.Tanh, bias=0.0, scale=1.0)
    V.tensor_tensor(out=tst3[:, :, 0:8], in0=th, in1=mask01, op=Alu.mult)
    V.tensor_scalar(tst3[:, :, 32:40], mask01, -1.0, BIGNEG, Alu.add, Alu.mult)
    gfm = nodep.tile([8, N], F32, name=f"gfm_{rep}", tag="gfm")
    qfm = nodep.tile([8, N], F32, name=f"qfm_{rep}", tag="qfm")
    mfm = nodep.tile([8, N], F32, name=f"mfm_{rep}", tag="mfm")
    for t in range(4):
        tq = psum1.tile([96, 128], F32, name="tq", tag="small")
        T.transpose(tq, tst3[:, t, :], C["identT"])
        S.activation(gfm[:, 128 * t:128 * (t + 1)], tq[0:8, :], Act.Copy,
                     bias=0.0, scale=1.0)
        S.activation(qfm[:, 128 * t:128 * (t + 1)], tq[32:40, :], Act.Copy,
                     bias=0.0, scale=1.0)
        S.activation(mfm[:, 128 * t:128 * (t + 1)], tq[64:72, :], Act.Copy,
                     bias=0.0, scale=1.0)
    gb = psum.tile([128, N], F32, name="gb", tag="zc")
    T.matmul(gb, mmdt(C["ones16b_stat"]), mmdt(gfm), start=True, stop=True)
    hp = nodep.tile([128, N], F32, name=f"hp_{rep}", tag="hp")
    V.tensor_tensor(out=hp, in0=h1, in1=gb, op=Alu.mult)
    dbg("hp", hp)
    qb = psum.tile([128, N], F32, name="qb", tag="zc")
    T.matmul(qb, mmdt(C["ones16b_stat"]), mmdt(qfm), start=True, stop=True)
    hq = nodep.tile([128, N + 1, 1], F32, name=f"hq_{rep}", tag="hq")
    V.tensor_tensor(out=hq[:, 0:N, 0], in0=hp, in1=qb, op=Alu.add)
    V.memset(hq[:, N:N + 1, 0], -BIGNEG)
    dbg("hq", hq[:, 0:N, 0])
    if STOP_STAGE == "hq":
        return stop_dma(hq[0:1, 0:1, 0])

    # ================= Layer 2 =================
    D0b, D1b = edge_layer(1, hq, C["t2vec"], "L2")
    if STOP_STAGE == "l2edge":
        return stop_dma(D0b[0:1, 0:1])
    aggr2 = aggr_from_D(D0b, D1b, 1)
    dbg("aggr2", aggr2)
    u2 = nodep.tile([128, N], F32, name=f"u2_{rep}", tag="u2")
    V.scalar_tensor_tensor(out=u2, in0=aggr2, scalar=EPS, in1=hp,
                           op0=Alu.add, op1=Alu.add)
    y2q2 = mlp(u2, C["w2a_stat"], C["b2a_vec"], C["g2_vec"], C["be2_vec"],
               C["w2bh_stat"], 1)
    h2 = [nodep.tile([128, N], F16, name=f"h2{sl}_{rep}", tag=f"h2{sl}")
          for sl in range(2)]
    for sl in range(2):
        S.activation(h2[sl], y2q2[sl], Act.Relu, bias=C["b2b_vec"], scale=1.0)
    dbg("h2a", h2[0]); dbg("h2b", h2[1])
    if STOP_STAGE == "mlp2":
        return stop_dma(h2[0][0:1, 0:1])

    # ================= pooling + head =================
    pooled = []
    for sl, statname in ((0, "maskbc_statA"), (1, "maskbc_statB")):
        mb2 = psum.tile([128, N], F32, name=f"mbp{sl}", tag="zc")
        T.matmul(mb2, mmdt(C[statname]), mmdt(mfm), start=True, stop=True)
        mbh = nodep.tile([128, N], F16, name=f"mbh{sl}_{rep}", tag=f"mbh{sl}")
        S.activation(mbh, mb2, Act.Copy, bias=0.0, scale=1.0)
        pl = nodep.tile([128, 1], F32, name=f"pl{sl}_{rep}", tag=f"pl{sl}")
        scratch = work.tile([128, N], F16, name="plscratch", tag="plscratch", bufs=1)
        V.scalar_tensor_tensor(out=scratch, in0=h2[sl], scalar=1.0, in1=mbh,
                               op0=Alu.mult, op1=Alu.mult, accum_out=pl)
        pooled.append(pl)
    P8 = psum1.tile([32, G], F32, name="P8", tag="small")
    for g in range(G):
        sl, gg = g // 4, g % 4
        T.matmul(P8[:, g:g + 1],
                 mmdt(C["selk_stat"][:, 32 * gg:32 * gg + 32]),
                 mmdt(pooled[sl]), start=True, stop=True,
                 skip_group_check=True)
    p8s = nodep.tile([32, G], F32, name=f"p8s_{rep}", tag="p8s")
    S.activation(p8s, P8, Act.Copy, bias=0.0, scale=1.0)
    dbg("pooled", p8s)
    a1p = psum.tile([128, G], F32, name="a1p", tag="zc")
    T.matmul(a1p, mmdt(C["wa_stat"]), mmdt(p8s), start=True, stop=True)
    a1 = nodep.tile([128, G], F32, name=f"a1_{rep}", tag="a1")
    S.activation(a1, a1p, Act.Relu, bias=C["ba_vec"], scale=1.0)
    op = psum1.tile([4, G], F32, name="op", tag="small")
    T.matmul(op, mmdt(C["wo_stat"]), mmdt(a1), start=True, stop=True)
    ot = nodep.tile([4, G], F32, name=f"ot_{rep}", tag="ot")
    S.activation(ot, op, Act.Tanh, bias=C["bo2_vec"], scale=1.0)
    nc.sync.dma_start(out=out_ext, in_=ot)


# ----------------------------------------------------------------------------
# Self-contained entry point: kernel(**inputs) -> [64, 4] float32
# ----------------------------------------------------------------------------
import jax as _jax
from jax.sharding import Mesh as _Mesh, PartitionSpec as _PartitionSpec
from jax.experimental.shard_map import shard_map as _shard_map

_COMPILED = {}


def _build_and_jit():
    """Re-create the jitted executable on every call: re-executing a loaded
    NEFF leaves device state (semaphores) behind and corrupts the second run,
    so each kernel() invocation gets a fresh executable (BIR->NEFF is
    disk-cached, so this costs seconds, not a recompile)."""
    from concourse import bass2jax
    from concourse.bass2jax import _bass_exec_p, partition_id_tensor

    if "nc" in _COMPILED:
        nc = _COMPILED["nc"]
    else:
        nc = build_nc()
        _COMPILED["nc"] = nc
    bass2jax.install_neuronx_cc_hook()
    partition_name = (nc.partition_id_tensor.name
                      if nc.partition_id_tensor else None)
    in_names, out_names, out_avals, zero_outs = [], [], [], []
    for alloc in nc.m.functions[0].allocations:
        if not isinstance(alloc, mybir.MemoryLocationSet):
            continue
        nm = alloc.memorylocations[0].name
        if alloc.kind == "ExternalInput":
            if nm != partition_name:
                in_names.append(nm)
        elif alloc.kind == "ExternalOutput":
            out_names.append(nm)
            out_avals.append(_jax.core.ShapedArray(
                tuple(alloc.tensor_shape), mybir.dt.np(alloc.dtype)))
            zero_outs.append(np.zeros(tuple(alloc.tensor_shape),
                                      mybir.dt.np(alloc.dtype)))
    n_params = len(in_names)
    n_outs = len(out_avals)
    in_names_all = in_names + out_names
    if partition_name is not None:
        in_names_all.append(partition_name)
    donate = tuple(range(n_params, n_params + n_outs))

    def _body(*args):
        operands = list(args)
        if partition_name is not None:
            operands.append(partition_id_tensor())
        return tuple(_bass_exec_p.bind(
            *operands, out_avals=tuple(out_avals),
            in_names=tuple(in_names_all), out_names=tuple(out_names),
            lowering_input_output_aliases=(), sim_require_finite=True,
            sim_require_nnan=True, nc=nc))

    devices = _jax.devices()[:8]
    mesh = _Mesh(np.asarray(devices), ("core",))
    in_specs = (_PartitionSpec("core"),) * (n_params + n_outs)
    out_specs = (_PartitionSpec("core"),) * len(out_names)
    sharded = _jax.jit(
        _shard_map(_body, mesh=mesh, in_specs=in_specs, out_specs=out_specs,
                   check_rep=False),
        donate_argnums=donate, keep_unused=True)
    return (sharded, in_names, out_names, zero_outs)


def kernel(**inputs):
    """Full-input GNN forward on 8 TRN2 NeuronCores; returns [64, 4] f32."""
    sharded, in_names, out_names, zero_outs = _build_and_jit()
    core_maps = prep_inputs(inputs)
    concat_in = [np.concatenate([core_maps[c][nm] for c in range(8)], axis=0)
                 for nm in in_names]
    concat_zero = [np.zeros((8 * z.shape[0], *z.shape[1:]), z.dtype)
                   for z in zero_outs]
    out_arrs = sharded(*concat_in, *concat_zero)
    oi = out_names.index("out")
    full = np.asarray(out_arrs[oi]).reshape(8, 4, G)
    return np.concatenate([full[c].T for c in range(8)], axis=0)



# revision 53
# speedup vs baseline: 1.0073x; 1.0073x over previous
"""GNN (GENConv x2 + TopK pool) Bass/Tile kernel for TRN2, data-parallel over
8 NeuronCores (8 graphs per core).

Per-core layout conventions ("fm" = feature-major packed):
  - Edge tensors:  [128 = 16feat x 8graph, 16384 slots]  (dst-sorted per graph)
    row 16*g + f holds feature f of graph g; free axis = slot (graph-local).
  - Node tensors:  [128 = 16f x 8g, 512 nodes]  (e.g. h, hq, aggr)
    or [128 = 32f x 4g, 512] for 32-dim stages (halves A: graphs 0-3, B: 4-7).
  - Segment sums via chunked f32 prefix scan + boundary gather (ap_gather) in
    2 waves of 8192 slots to bound SBUF.
"""

import numpy as np
from contextlib import ExitStack

import concourse.bass as bass
import concourse.bacc as bacc
import concourse.mybir as mybir
import concourse.tile as tile
from concourse import library_config

F32 = mybir.dt.float32
F32R = mybir.dt.float32r
F16 = mybir.dt.float16
I16 = mybir.dt.int16
I8 = mybir.dt.int8

G = 8          # graphs per core
N = 512        # nodes per graph
EG = 16384     # edges per graph
EF = 16        # edge/node feature dim after encode
XF = 64        # input node feature dim
K = 256        # topk keep
SLOTS = 18432  # padded slots per graph (each node run padded to mult of 4)
Q4 = SLOTS // 4
HQ = Q4 // 2   # boundary-gather table split point
CHUNK = 1024   # slots per elementwise chunk
NCHUNK = SLOTS // CHUNK
PCH = 512      # slots per psum chunk (one bank)
NEND = 576     # padded end-list length (513 used)
EPS = 1e-7
BIGNEG = 2.0e9

USE_F32R = False   # set True once f32r numerics verified on HW
STOP_STAGE = None  # for HW bisects: "l1edge", "aggr1", "mlp1", "rank", "hq", "l2edge", "aggr2", "mlp2"


def mmdt(ap):
    return ap.bitcast(F32R) if USE_F32R else ap


# ----------------------------------------------------------------------------
# Host-side preprocessing: full inputs -> per-core named arrays
# ----------------------------------------------------------------------------

def prep_inputs(inputs: dict) -> list[dict]:
    x = np.asarray(inputs["x"], np.float32)            # [B*N, 64]
    ei = np.asarray(inputs["edge_index"])              # [2, E] int64
    ea = np.asarray(inputs["edge_attr"], np.float32)   # [E, 16]
    B = 64
    assert x.shape == (B * N, XF)
    assert ea.shape == (B * EG, EF)

    src_g = (ei[0] % N).astype(np.int64)
    dst_g = (ei[1] % N).astype(np.int64)
    graph_of_edge = (ei[0] // N).astype(np.int64)
    assert np.array_equal(graph_of_edge, np.repeat(np.arange(B), EG)), \
        "edge blocks not per-graph; prep assumes reference setup_inputs layout"
    assert np.array_equal(ei[0] // N, ei[1] // N)

    def lin(name):
        return np.asarray(inputs[name], np.float32)

    W_ne, b_ne = lin("W_ne"), lin("b_ne")
    W_ee, b_ee = lin("W_ee"), lin("b_ee")
    W1a, b1a, g1, be1 = lin("W1a"), lin("b1a"), lin("g1"), lin("be1")
    W1b, b1b = lin("W1b"), lin("b1b")
    W2a, b2a, g2, be2 = lin("W2a"), lin("b2a"), lin("g2"), lin("be2")
    W2b, b2b = lin("W2b"), lin("b2b")
    Wa, ba, Wo, bo = lin("Wa"), lin("ba"), lin("Wo"), lin("bo")
    w_pool = lin("w_pool")
    wp = w_pool / np.linalg.norm(w_pool)
    t1 = np.float32(inputs["t1"])
    t2 = np.float32(inputs["t2"])

    cst = {}
    wne = np.zeros((128, 4 * 128), np.float32)
    for p in range(4):
        for a in range(2):
            gg = 2 * p + a
            wne[64 * a:64 * a + XF, 128 * p + 16 * gg:128 * p + 16 * gg + EF] = W_ne
    cst["wne_stat"] = wne
    cst["bne_vec"] = np.tile(b_ne, G)[:, None].astype(np.float32)
    wee = np.zeros((128, 128), np.float32)
    for g in range(G):
        wee[16 * g:16 * g + EF, 16 * g:16 * g + EF] = W_ee
    cst["weeh_stat"] = wee.astype(np.float16)
    cst["bee_vec"] = np.tile(b_ee, G)[:, None].astype(np.float32)
    cst["ident"] = np.eye(128, dtype=np.float32)
    cst["identT"] = np.eye(128, dtype=np.float32)
    cst["identh"] = np.eye(128, dtype=np.float16)
    cst["t1vec"] = np.full((128, 1), t1, np.float32)
    cst["t2vec"] = np.full((128, 1), t2, np.float32)
    w1a = np.zeros((64, 128), np.float32)
    for gg in range(4):
        w1a[16 * gg:16 * gg + 16, 32 * gg:32 * gg + 32] = W1a
    cst["w1a_stat"] = np.vstack([w1a, w1a])
    cst["b1a_vec"] = np.tile(b1a, 4)[:, None].astype(np.float32)
    ones32 = np.zeros((128, 4), np.float32)
    for gg in range(4):
        ones32[32 * gg:32 * gg + 32, gg] = 1.0 / 32.0
    cst["ones32_stat"] = ones32
    onesb32 = np.zeros((4, 128), np.float32)
    for gg in range(4):
        onesb32[gg, 32 * gg:32 * gg + 32] = 1.0
    cst["onesb32_stat"] = onesb32
    cst["g1_vec"] = np.tile(g1, 4)[:, None].astype(np.float32)
    cst["be1_vec"] = np.tile(be1, 4)[:, None].astype(np.float32)
    w1b = np.zeros((64, 32), np.float32)
    for gg in range(2):
        w1b[32 * gg:32 * gg + 32, 16 * gg:16 * gg + 16] = W1b
    cst["w1b_stat"] = np.vstack([w1b, w1b])
    cst["b1b_vec"] = np.tile(b1b, G)[:, None].astype(np.float32)
    wpool = np.zeros((128, 8), np.float32)
    for g in range(G):
        wpool[16 * g:16 * g + EF, g] = wp
    cst["wpool_stat"] = wpool
    ones16b = np.zeros((8, 128), np.float32)
    for g in range(G):
        ones16b[g, 16 * g:16 * g + EF] = 1.0
    cst["ones16b_stat"] = ones16b
    cst["ones1x128"] = np.ones((1, 128), np.float32)
    cst["ones8x128"] = np.ones((8, 128), np.float32)
    onesel = np.zeros((8, 8 * 128), np.float32)
    for g in range(8):
        onesel[g, 128 * g:128 * (g + 1)] = 1.0
    cst["onesel_stat"] = onesel
    w2a = np.zeros((64, 128), np.float32)
    for gg in range(4):
        w2a[16 * gg:16 * gg + 16, 32 * gg:32 * gg + 32] = W2a
    cst["w2a_stat"] = np.vstack([w2a, w2a])
    cst["b2a_vec"] = np.tile(b2a, 4)[:, None].astype(np.float32)
    cst["g2_vec"] = np.tile(g2, 4)[:, None].astype(np.float32)
    cst["be2_vec"] = np.tile(be2, 4)[:, None].astype(np.float32)
    w2b = np.zeros((64, 64), np.float32)
    for gg in range(2):
        w2b[32 * gg:32 * gg + 32, 32 * gg:32 * gg + 32] = W2b
    cst["w2b_stat"] = np.vstack([w2b, w2b])
    cst["b2b_vec"] = np.tile(b2b, 4)[:, None].astype(np.float32)
    mbA = np.zeros((8, 128), np.float32)
    mbB = np.zeros((8, 128), np.float32)
    for g in range(4):
        mbA[g, 32 * g:32 * g + 32] = 1.0
        mbB[g + 4, 32 * g:32 * g + 32] = 1.0
    cst["maskbc_statA"] = mbA
    cst["maskbc_statB"] = mbB
    selk = np.zeros((128, 4 * 32), np.float32)
    for gg in range(4):
        selk[32 * gg:32 * gg + 32, 32 * gg:32 * gg + 32] = np.eye(32) / K
    cst["selk_stat"] = selk
    cst["wa_stat"] = Wa.astype(np.float32)
    cst["ba_vec"] = ba[:, None].astype(np.float32)
    cst["wo_stat"] = Wo.astype(np.float32)
    cst["bo2_vec"] = bo[:, None].astype(np.float32)
    cst["lneps_vec"] = np.full((4, 1), 1e-5, np.float32)
    cst["ones32h_stat"] = ones32.astype(np.float16)
    cst["w1bh_stat"] = cst["w1b_stat"].astype(np.float16)
    cst["w2bh_stat"] = cst["w2b_stat"].astype(np.float16)
    cst["big8_vec"] = np.full((128, 1), 1e8, np.float32)
    m1a = (w1a @ ones32).astype(np.float32)
    cst["m1a_stat"] = np.vstack([m1a, m1a])                      # [128, 4]
    cst["m1bias"] = (ones32.T @ np.tile(b1a, 4)[:, None]).astype(np.float32)
    m2a = (w2a @ ones32).astype(np.float32)
    cst["m2a_stat"] = np.vstack([m2a, m2a])
    cst["m2bias"] = (ones32.T @ np.tile(b2a, 4)[:, None]).astype(np.float32)

    core_maps = []
    for core in range(8):
        m = dict(cst)
        gsl = slice(core * G, (core + 1) * G)
        xt = np.zeros((128, 4 * 512), np.float32)
        xs = x.reshape(B, N, XF)[gsl]
        for p in range(4):
            for a in range(2):
                xt[64 * a:64 * a + XF, 512 * p:512 * (p + 1)] = xs[2 * p + a].T
        m["xT"] = xt

        attrT = np.zeros((128, SLOTS), np.float16)
        srcidx = np.full((128, SLOTS // 16), 512, np.int16)
        end4a = np.zeros((128, NEND // 16), np.int16)
        end4b = np.zeros((128, NEND // 16), np.int16)
        selm = np.zeros((128, NEND), np.int8)
        npad = np.zeros((128, N), np.float32)
        for gl in range(G):
            gid = core * G + gl
            s_l = src_g[gid * EG:(gid + 1) * EG]
            d_l = dst_g[gid * EG:(gid + 1) * EG]
            order = np.argsort(d_l, kind="stable")
            ds = d_l[order]
            ea_s = ea[gid * EG:(gid + 1) * EG][order]  # [EG, 16] dst-sorted
            ss = s_l[order]
            ends = np.searchsorted(ds, np.arange(N), side="right")
            starts = np.concatenate([[0], ends[:-1]])
            deg = ends - starts
            pdeg = (deg + 3) // 4 * 4
            off4 = np.concatenate([[0], np.cumsum(pdeg)])
            assert off4[-1] <= SLOTS
            # scatter real edges into padded positions
            pos = off4[ds] + (np.arange(EG) - starts[ds])
            at = np.zeros((SLOTS, EF), np.float16)
            at[pos] = ea_s.astype(np.float16)
            sp = np.full(SLOTS, 512, np.int16)
            sp[pos] = ss.astype(np.int16)
            attrT[16 * gl:16 * gl + EF, :] = at.T
            srcidx[16 * gl:16 * gl + 16, :] = sp.reshape(SLOTS // 16, 16).T
            e4 = np.zeros(NEND, np.int64)
            e4[:513] = off4 // 4
            e4[513:] = e4[512]
            ea4 = np.minimum(e4, HQ).astype(np.int16)
            eb4 = np.clip(e4 - HQ, 0, Q4 - HQ).astype(np.int16)
            end4a[16 * gl:16 * gl + 16, :] = ea4.reshape(NEND // 16, 16).T
            end4b[16 * gl:16 * gl + 16, :] = eb4.reshape(NEND // 16, 16).T
            selm[16 * gl:16 * gl + 16, :] = (e4 <= HQ).astype(np.int8)[None, :]
            npad[16 * gl:16 * gl + 16, :] = (pdeg - deg).astype(np.float32)[None, :]
        m["attrT"] = attrT
        m["srcidx"] = srcidx
        m["end4a"] = end4a
        m["end4b"] = end4b
        m["selm"] = selm
        m["npad"] = npad
        blob = np.zeros((128, CBLOB_BYTES), np.uint8)
        for name, shape, dt, off in CONST_SPECS:
            arr = m[name]
            bv = arr.view(np.uint8).reshape(arr.shape[0], -1)
            blob[:arr.shape[0], off:off + bv.shape[1]] = bv
        core_maps.append({"cblob": blob, "attrT": m["attrT"],
                          "ident": m["ident"],
                          "xT": m["xT"], "srcidx": m["srcidx"]})
    return core_maps


_CSPEC_RAW = [
    ("wne_stat", [128, 4 * 128], F32),
    ("bne_vec", [128, 1], F32),
    ("end4a", [128, NEND // 16], I16),
    ("end4b", [128, NEND // 16], I16),
    ("npad", [128, N], F32),
    ("weeh_stat", [128, 128], F16),
    ("identh", [128, 128], F16),
    ("bee_vec", [128, 1], F32),
    ("identT", [128, 128], F32),
    ("t1vec", [128, 1], F32),
    ("t2vec", [128, 1], F32),
    ("big8_vec", [128, 1], F32),
    ("w1a_stat", [128, 128], F32),
    ("b1a_vec", [128, 1], F32),
    ("ones32_stat", [128, 4], F32),
    ("onesb32_stat", [4, 128], F32),
    ("g1_vec", [128, 1], F32),
    ("be1_vec", [128, 1], F32),
    ("w1b_stat", [128, 32], F32),
    ("b1b_vec", [128, 1], F32),
    ("wpool_stat", [128, 8], F32),
    ("ones16b_stat", [8, 128], F32),
    ("ones1x128", [1, 128], F32),
    ("ones8x128", [8, 128], F32),
    ("onesel_stat", [8, 8 * 128], F32),
    ("w2a_stat", [128, 128], F32),
    ("b2a_vec", [128, 1], F32),
    ("g2_vec", [128, 1], F32),
    ("be2_vec", [128, 1], F32),
    ("w2b_stat", [128, 64], F32),
    ("b2b_vec", [128, 1], F32),
    ("maskbc_statA", [8, 128], F32),
    ("maskbc_statB", [8, 128], F32),
    ("selk_stat", [128, 4 * 32], F32),
    ("wa_stat", [32, 128], F32),
    ("ba_vec", [128, 1], F32),
    ("wo_stat", [128, 4], F32),
    ("bo2_vec", [4, 1], F32),
    ("lneps_vec", [4, 1], F32),
    ("ones32h_stat", [128, 4], F16),
    ("w1bh_stat", [128, 32], F16),
    ("w2bh_stat", [128, 64], F16),
    ("selm", [128, NEND], I8),
    ("m1a_stat", [128, 4], F32),
    ("m1bias", [4, 1], F32),
    ("m2a_stat", [128, 4], F32),
    ("m2bias", [4, 1], F32),
]

def _mk_const_specs():
    specs = []
    off = 0
    import numpy as _np
    for nm, shape, dt in _CSPEC_RAW:
        nbytes = int(_np.prod(shape[1:])) * mybir.dt.size(dt) if len(shape) > 1 else mybir.dt.size(dt)
        nbytes = int(_np.prod(shape[1:])) * mybir.dt.size(dt)
        specs.append((nm, shape, dt, off))
        off += (nbytes + 127) // 128 * 128
    return specs, off

CONST_SPECS, CBLOB_BYTES = _mk_const_specs()

INPUT_SPECS = [
    ("cblob", [128, None], mybir.dt.uint8),
    ("attrT", [128, SLOTS], F16),
    ("ident", [128, 128], F32R),
    ("xT", [128, 4 * 512], F32),
    ("srcidx", [128, SLOTS // 16], I16),
]
INPUT_SPECS[0] = ("cblob", [128, CBLOB_BYTES], mybir.dt.uint8)


# ----------------------------------------------------------------------------
# Device graph
# ----------------------------------------------------------------------------

def build_nc(debug_keys=(), n_rep=1):
    nc = bacc.Bacc(None, target_bir_lowering=False, debug=False)
    A = {}
    for name, shape, dt in INPUT_SPECS:
        A[name] = nc.declare_dram_parameter(name, shape, dt, isOutput=False)[:]
    out_ext = nc.declare_dram_parameter("out", [4, G], F32, isOutput=True)[:]
    dbg_ext = {}
    dbg_shapes = {
        "hT": ((128, N), F32), "hsrc0": ((128, CHUNK), F32),
        "m0": ((128, CHUNK), F16), "ev0": ((128, CHUNK), F16),
        "S0w0": ((128, Q4 + 1), F32), "S1w0": ((128, Q4 + 1), F32),
        "G0": ((128, NEND), F32), "G1": ((128, NEND), F32),
        "aggr1": ((128, N), F32), "h1": ((128, N), F32),
        "score": ((8, N), F32), "snode": ((128, 32), F32),
        "rk": ((128, 32), F32), "mask": ((128, 32), F32),
        "hq": ((128, N), F32), "hp": ((128, N), F32),
        "aggr2": ((128, N), F32), "h2a": ((128, N), F16),
        "h2b": ((128, N), F16), "pooled": ((32, 8), F32),
        "u1": ((128, N), F32), "y1sA": ((128, N), F32), "r1A": ((128, N), F32),
    }
    for key in debug_keys:
        shape, dt = dbg_shapes[key]
        dbg_ext[key] = nc.declare_dram_parameter(
            "dbg_" + key, list(shape), dt, isOutput=True)[:]

    with tile.TileContext(nc) as tc, ExitStack() as ctx:
        consts = ctx.enter_context(tc.tile_pool(name="consts", bufs=1))
        nodep = ctx.enter_context(tc.tile_pool(name="nodep", bufs=1))
        scanp = ctx.enter_context(tc.tile_pool(name="scanp", bufs=1))
        work = ctx.enter_context(tc.tile_pool(name="work", bufs=2))
        psum1 = ctx.enter_context(tc.tile_pool(name="psum1", bufs=2, space="PSUM"))
        psume = ctx.enter_context(tc.tile_pool(name="psume", bufs=2, space="PSUM"))
        psumq = ctx.enter_context(tc.tile_pool(name="psumq", bufs=2, space="PSUM"))
        psum = psume
        psumw = psume

        nc.gpsimd.load_library(library_config.ap_gather)

        C = {}
        cb = consts.tile([128, CBLOB_BYTES], mybir.dt.uint8, name="cblob_sb")
        _split = CONST_SPECS[12][3]
        nc.sync.dma_start(out=cb[:, :_split], in_=A["cblob"][:, :_split])
        for _n, _shape, _dt in (("srcidx", [128, SLOTS // 16], I16),
                                ("ident", [128, 128], F32R)):
            _t = consts.tile(_shape, _dt, name="c_" + _n)
            nc.scalar.dma_start(out=_t, in_=A[_n])
            C[_n] = _t
        _rest = CBLOB_BYTES - _split
        _step = (_rest + 3) // 4
        for _k in range(4):
            _a = _split + _k * _step
            _b = min(_split + (_k + 1) * _step, CBLOB_BYTES)
            if _a < _b:
                nc.scalar.dma_start(out=cb[:, _a:_b], in_=A["cblob"][:, _a:_b])
        for name, shape, dt, off in CONST_SPECS:
            nbytes = int(np.prod(shape[1:])) * mybir.dt.size(dt)
            ap = cb[:shape[0], off:off + nbytes].bitcast(dt)
            if len(shape) == 3:
                ap = ap.rearrange("p (a b) -> p a b", b=shape[2])
            C[name] = ap

        dbg_keys_set = set(debug_keys)

        def dbg(key, ap):
            if key in dbg_keys_set:
                nc.sync.dma_start(out=dbg_ext[key], in_=ap)

        for rep in range(n_rep):
            run_once(nc, tc, A, C, out_ext, dbg, nodep, scanp, work,
                     psum, psum1, psume, psumq, psumw, rep)
    nc.compile()
    return nc


def run_once(nc, tc, A, C, out_ext, dbg, nodep, scanp, work, psum, psum1,
             psume, psumq, psumw, rep):
    V = nc.vector
    S = nc.scalar
    T = nc.tensor
    Alu = mybir.AluOpType
    Act = mybir.ActivationFunctionType

    # ================= node encode: hT = x @ W_ne + b_ne =================
    hraw = psum.tile([128, N], F32, name="hraw", tag="zc")
    for half in range(2):
        xts = work.tile([128, 1024], F32, name=f"xts{half}", tag="xts")
        nc.sync.dma_start(out=xts, in_=A["xT"][:, 1024 * half:1024 * (half + 1)])
        for pp in range(2):
            p = 2 * half + pp
            T.matmul(hraw, mmdt(C["wne_stat"][:, 128 * p:128 * (p + 1)]),
                     mmdt(xts[:, 512 * pp:512 * (pp + 1)]),
                     start=(p == 0), stop=(p == 3))
    hT = nodep.tile([128, N + 1, 1], F32, name="hT")
    S.activation(hT[:, 0:N, 0], hraw, Act.Identity, bias=C["bne_vec"], scale=1.0)
    V.memset(hT[:, N:N + 1, 0], -BIGNEG)  # sentinel column for pad slots
    dbg("hT", hT[:, 0:N, 0])
    zeros512 = nodep.tile([128, PCH], F32, name="zeros512")
    V.memset(zeros512, 0.0)

    # ================= shared edge-phase machinery =================
    # Edge slots are dst-sorted with每 node's run padded to a multiple of 4
    # (pad slots gather the sentinel column -> mc=0, ev=1 in L1 / 0 in L2).
    # Per chunk: zc accumulates in PSUM (wee@attr + ident@hsrc), then 4:1
    # block sums via strided identity matmuls -> short scan -> small gather.
    def edge_layer(layer, table, tvec, dbg_pref):
        S4 = [scanp.tile([128, Q4 + 1, 1], F32, name=f"S4{ti}_{layer}_{rep}",
                         tag=f"S4{ti}") for ti in range(2)]
        V.memset(S4[0][:, 0:1, 0], 0.0)
        V.memset(S4[1][:, 0:1, 0], 0.0)

        for cc in range(NCHUNK):
            base = cc * CHUNK
            attrc = work.tile([128, CHUNK], F16, name="attrc", tag="attrc", bufs=3)
            nc.sync.dma_start(out=attrc, in_=A["attrT"][:, base:base + CHUNK])
            hsrc = work.tile([128, CHUNK, 1], F32, name="hsrc", tag="hsrc", bufs=3)
            j0 = base // 16
            nc.gpsimd.ap_gather(
                hsrc, table, C["srcidx"][:, j0:j0 + CHUNK // 16],
                channels=128, num_elems=N + 1, d=1, num_idxs=CHUNK)
            eap = psume.tile([128, CHUNK], F32, name="zc", tag="zc")
            for s in range(CHUNK // PCH):
                T.matmul(eap[:, PCH * s:PCH * (s + 1)], C["weeh_stat"],
                         attrc[:, PCH * s:PCH * (s + 1)],
                         start=True, stop=True)
            zc = work.tile([128, CHUNK], F32, name="zcs", tag="zcs", bufs=3)
            V.tensor_tensor(out=zc, in0=hsrc[:, :, 0], in1=eap, op=Alu.add)
            mc = work.tile([128, CHUNK], F16, name="mc", tag="mc", bufs=3)
            S.activation(mc, zc, Act.Relu, bias=C["bee_vec"], scale=1.0)
            if layer == 0 and base == 0:
                dbg("hsrc0", hsrc[:, :, 0])
            # evc | evmc packed side by side so the 4:1 reduce runs as one
            # 4-matmul group over both arrays.
            evp = work.tile([128, 2 * CHUNK], F16, name="evp", tag="evp",
                            bufs=3)
            evc = evp[:, 0:CHUNK]
            if layer == 0:
                S.activation(evc, mc, Act.Exp, bias=0.0, scale=tvec)
            else:
                sgn = work.tile([128, CHUNK], F16, name="sgn", tag="sgn")
                S.activation(sgn, zc, Act.Sign, bias=C["big8_vec"], scale=1.0)
                ev0 = work.tile([128, CHUNK], F16, name="ev0t", tag="ev0t",
                                bufs=2)
                S.activation(ev0, mc, Act.Exp, bias=0.0, scale=tvec)
                V.scalar_tensor_tensor(out=evc, in0=sgn, scalar=0.0, in1=ev0,
                                       op0=Alu.max, op1=Alu.mult)
            V.tensor_tensor(out=evp[:, CHUNK:2 * CHUNK], in0=evc, in1=mc,
                            op=Alu.mult)
            if layer == 0 and base == 0:
                dbg("m0", mc); dbg("ev0", evc)
            lo = cc * (CHUNK // 4)
            q4 = psumq.tile([128, CHUNK // 2], F32, name="q4", tag="q4")
            a3 = evp.rearrange("p (a b) -> p a b", b=4)
            for j in range(4):
                T.matmul(q4, C["identh"], a3[:, :, j],
                         start=(j == 0), stop=(j == 3))
            for ti in range(2):
                V.tensor_tensor_scan(
                    out=S4[ti][:, lo + 1:lo + 1 + CHUNK // 4, 0],
                    data0=q4[:, ti * (CHUNK // 4):(ti + 1) * (CHUNK // 4)],
                    data1=zeros512[:, 0:CHUNK // 4],
                    initial=S4[ti][:, lo:lo + 1, 0],
                    op0=Alu.add, op1=Alu.add)

        D = []
        GA = []
        for ti in range(2):
            # first-half gather only needs chunks 0..8 -> fires mid-layer
            gwa = nodep.tile([128, NEND, 1], F32,
                             name=f"gwa{ti}_{layer}_{rep}", tag=f"gwa{ti}")
            nc.gpsimd.ap_gather(
                gwa, S4[ti][:, 0:HQ + 1, :], C["end4a"],
                channels=128, num_elems=HQ + 1, d=1, num_idxs=NEND)
            GA.append(gwa)
        for ti in range(2):
            gwb = nodep.tile([128, NEND, 1], F32,
                             name=f"gwb{ti}_{layer}_{rep}", tag=f"gwb{ti}")
            nc.gpsimd.ap_gather(
                gwb, S4[ti][:, HQ:Q4 + 1, :], C["end4b"],
                channels=128, num_elems=Q4 - HQ + 1, d=1, num_idxs=NEND)
            gsel = nodep.tile([128, NEND], F32,
                              name=f"gsel{ti}_{layer}_{rep}", tag=f"gsel{ti}")
            V.select(gsel, C["selm"], GA[ti][:, :, 0], gwb[:, :, 0])
            dbg("G0" if ti == 0 else "G1", gsel)
            d = nodep.tile([128, N], F32, name=f"D{ti}_{layer}_{rep}",
                           tag=f"D{ti}")
            V.tensor_tensor(out=d, in0=gsel[:, 1:N + 1], in1=gsel[:, 0:N],
                            op=Alu.subtract)
            D.append(d)
        if layer == 0:
            d0c = nodep.tile([128, N], F32, name=f"D0c_{layer}_{rep}",
                             tag="D0c")
            V.tensor_tensor(out=d0c, in0=D[0], in1=C["npad"], op=Alu.subtract)
            D[0] = d0c
        return D

    def aggr_from_D(D0, D1, layer):
        dm = nodep.tile([128, N], F32, name=f"dm_{layer}_{rep}", tag="dm")
        V.tensor_scalar(dm, D0, 1e-16, None, Alu.max)
        rec = nodep.tile([128, N], F32, name=f"rec_{layer}_{rep}", tag="rec")
        V.reciprocal(rec, dm)
        ag = nodep.tile([128, N], F32, name=f"ag_{layer}_{rep}", tag="ag")
        V.tensor_tensor(out=ag, in0=D1, in1=rec, op=Alu.mult)
        return ag

    def mlp(uin, wa_stat, ba_vec, gvec, bevec, wb_stat, layer):
        N2 = 2 * N
        y1p = psumw.tile([128, 2, N], F32, name=f"y1p_{layer}", tag="zc")
        for half in range(2):
            T.matmul(y1p[:, half, :], mmdt(wa_stat[64 * half:64 * half + 64, :]),
                     mmdt(uin[64 * half:64 * half + 64, :]),
                     start=True, stop=True)
        y1pf = y1p.rearrange("p a b -> p (a b)")
        y1s = nodep.tile([128, N2], F32, name=f"y1s_{layer}_{rep}", tag="y1s")
        S.activation(y1s, y1pf, Act.Identity, bias=ba_vec, scale=1.0)
        if layer == 0:
            dbg("y1sA", y1s[:, 0:N])
        mstat = C["m1a_stat"] if layer == 0 else C["m2a_stat"]
        mbias = C["m1bias"] if layer == 0 else C["m2bias"]
        mp = psumw.tile([4, 2, N], F32, name=f"mp_{layer}", tag="zc")
        for half in range(2):
            T.matmul(mp[:, half, :], mmdt(mstat[64 * half:64 * half + 64, :]),
                     mmdt(uin[64 * half:64 * half + 64, :]),
                     start=True, stop=True)
        ms = nodep.tile([4, N2], F32, name=f"ms_{layer}_{rep}", tag="st4", bufs=2)
        S.activation(ms, mp.rearrange("p a b -> p (a b)"), Act.Identity,
                     bias=mbias, scale=1.0)
        mb = psumw.tile([128, 2, N], F32, name=f"mb_{layer}", tag="zc")
        for half in range(2):
            T.matmul(mb[:, half, :], mmdt(C["onesb32_stat"]),
                     mmdt(ms[:, half * N:half * N + N]), start=True, stop=True)
        yc = nodep.tile([128, N2], F32, name=f"yc_{layer}_{rep}", tag="yc")
        V.tensor_tensor(out=yc, in0=y1s, in1=mb.rearrange("p a b -> p (a b)"),
                        op=Alu.subtract)
        sq = nodep.tile([128, N2], F16, name=f"sq_{layer}_{rep}", tag="sq")
        S.activation(sq, yc, Act.Square, bias=0.0, scale=1.0)
        vp = psumw.tile([4, 2, N], F32, name=f"vp_{layer}", tag="zc")
        for half in range(2):
            T.matmul(vp[:, half, :], C["ones32h_stat"],
                     sq[:, half * N:half * N + N], start=True, stop=True)
        lnv = nodep.tile([4, N2], F32, name=f"lnv_{layer}_{rep}", tag="st4", bufs=2)
        S.activation(lnv, vp.rearrange("p a b -> p (a b)"), Act.Ln,
                     bias=C["lneps_vec"], scale=1.0)
        rstd = nodep.tile([4, N2], F32, name=f"rstd_{layer}_{rep}", tag="st4", bufs=2)
        S.activation(rstd, lnv, Act.Exp, bias=0.0, scale=-0.5)
        rb = psumw.tile([128, 2, N], F32, name=f"rb_{layer}", tag="zc")
        for half in range(2):
            T.matmul(rb[:, half, :], mmdt(C["onesb32_stat"]),
                     mmdt(rstd[:, half * N:half * N + N]), start=True, stop=True)
        vnorm = nodep.tile([128, N2], F32, name=f"vn_{layer}_{rep}", tag="vn")
        V.tensor_tensor(out=vnorm, in0=yc, in1=rb.rearrange("p a b -> p (a b)"),
                        op=Alu.mult)
        r1 = nodep.tile([128, N2], F16, name=f"r1_{layer}_{rep}", tag="r1")
        S.activation(r1, vnorm, Act.Relu, bias=bevec, scale=gvec)
        if layer == 0:
            dbg("r1A", r1[:, 0:N])
        M = wb_stat.shape[1]
        outs = [psum.tile([2 * M, N], F32, name=f"yqb{b}_{layer}", tag="zc")
                for b in range(2)]
        idx = 0
        for half in range(2):
            for q in range(2):
                bank, slot = divmod(idx, 2)
                T.matmul(outs[bank][slot * M:(slot + 1) * M, :],
                         wb_stat[64 * q:64 * q + 64, :],
                         r1[64 * q:64 * q + 64, half * N:half * N + N],
                         start=True, stop=True, skip_group_check=True)
                idx += 1
        return outs

    # ================= Layer 1 =================
    D0, D1 = edge_layer(0, hT, C["t1vec"], "")
    if STOP_STAGE == "l1edge":
        return stop_dma(D0[0:1, 0:1])
    aggr1 = aggr_from_D(D0, D1, 0)
    dbg("aggr1", aggr1)
    if STOP_STAGE == "aggr1":
        return stop_dma(aggr1[0:1, 0:1])
    u1 = nodep.tile([128, N], F32, name=f"u1_{rep}", tag="u1")
    V.scalar_tensor_tensor(out=u1, in0=aggr1, scalar=EPS, in1=hT[:, 0:N, 0],
                           op0=Alu.add, op1=Alu.add)
    dbg("u1", u1)
    y2q = mlp(u1, C["w1a_stat"], C["b1a_vec"], C["g1_vec"], C["be1_vec"],
              C["w1bh_stat"], 0)
    h1 = nodep.tile([128, N], F32, name=f"h1_{rep}", tag="h1")
    for b in range(2):
        S.activation(h1[64 * b:64 * (b + 1), :], y2q[b], Act.Relu,
                     bias=C["b1b_vec"][64 * b:64 * (b + 1), :], scale=1.0)
    dbg("h1", h1)
    if STOP_STAGE == "mlp1":
        return stop_dma(h1[0:1, 0:1])

    # ================= score / topk mask / gates =================
    scp = psum.tile([8, N], F32, name="scp", tag="zc")
    T.matmul(scp, C["wpool_stat"], h1, start=True, stop=True)
    scs = nodep.tile([8, N], F32, name=f"scs_{rep}", tag="scs")
    S.activation(scs, scp, Act.Copy, bias=0.0, scale=1.0)
    dbg("score", scs)
    snode = nodep.tile([128, 4, 8], F32, name=f"snode_{rep}", tag="snode")
    for t in range(4):
        tp = psum1.tile([128, 8], F32, name="tp", tag="small")
        T.transpose(tp, scs[:, 128 * t:128 * (t + 1)], C["identT"][0:8, 0:8])
        S.activation(snode[:, t, :], tp, Act.Copy, bias=0.0, scale=1.0)
    dbg("snode", snode.rearrange("p a b -> p (a b)"))
    sneg = nodep.tile([128, 4, 8], F32, name=f"sneg_{rep}", tag="sneg")
    V.tensor_scalar(sneg, snode, -1.0, None, Alu.mult)
    rk = nodep.tile([128, 4, 8], F32, name=f"rk_{rep}", tag="rk")
    sgnscratch = work.tile([128, N], F16, name="sgnscratch", tag="plscratch", bufs=1)
    gtscratch = work.tile([128, N], F16, name="gtscratch", tag="gtscratch", bufs=1)
    for g in range(G):
        sb = psum.tile([128, N], F32, name="sb", tag="zc")
        T.matmul(sb, mmdt(C["onesel_stat"][:, 128 * g:128 * (g + 1)]),
                 mmdt(scs), start=True, stop=True)
        # t 0/1 on Act (rk = sum of signs, keep iff rk <= -1);
        # t 2/3 on DVE (rk = count greater, keep iff rk <= 255.5)
        for t in range(2):
            S.activation(sgnscratch, sb, Act.Sign,
                         bias=sneg[:, t, g:g + 1], scale=1.0,
                         accum_out=rk[:, t, g:g + 1])
        for t in range(2, 4):
            V.tensor_scalar(gtscratch, sb, snode[:, t, g:g + 1], 0.0,
                            Alu.is_gt, Alu.add,
                            accum_out=rk[:, t, g:g + 1])
    dbg("rk", rk.rearrange("p a b -> p (a b)"))
    tst3 = nodep.tile([128, 4, 96], F32, name=f"tst3_{rep}", tag="tst3")
    mask01 = tst3[:, :, 64:72]
    V.tensor_scalar(mask01[:, 0:2, :], rk[:, 0:2, :], -1.0, None, Alu.is_le)
    V.tensor_scalar(mask01[:, 2:4, :], rk[:, 2:4, :], 255.5, None, Alu.is_le)
    th = nodep.tile([128, 4, 8], F32, name=f"th_{rep}", tag="th")
    S.activation(th, snode, Act.Tanh, bias=0.0, scale=1.0)
    V.tensor_tensor(out=tst3[:, :, 0:8], in0=th, in1=mask01, op=Alu.mult)
    V.tensor_scalar(tst3[:, :, 32:40], mask01, -1.0, BIGNEG, Alu.add, Alu.mult)
    gfm = nodep.tile([8, N], F32, name=f"gfm_{rep}", tag="gfm")
    qfm = nodep.tile([8, N], F32, name=f"qfm_{rep}", tag="qfm")
    mfm = nodep.tile([8, N], F32, name=f"mfm_{rep}", tag="mfm")
    for t in range(4):
        tq = psum1.tile([96, 128], F32, name="tq", tag="small")
        T.transpose(tq, tst3[:, t, :], C["identT"])
        S.activation(gfm[:, 128 * t:128 * (t + 1)], tq[0:8, :], Act.Copy,
                     bias=0.0, scale=1.0)
        S.activation(qfm[:, 128 * t:128 * (t + 1)], tq[32:40, :], Act.Copy,
                     bias=0.0, scale=1.0)
        S.activation(mfm[:, 128 * t:128 * (t + 1)], tq[64:72, :], Act.Copy,
                     bias=0.0, scale=1.0)
    gb = psum.tile([128, N], F32, name="gb", tag="zc")
    T.matmul(gb, mmdt(C["ones16b_stat"]), mmdt(gfm), start=True, stop=True)
    hp = nodep.tile([128, N], F32, name=f"hp_{rep}", tag="hp")
    V.tensor_tensor(out=hp, in0=h1, in1=gb, op=Alu.mult)
    dbg("hp", hp)
    qb = psum.tile([128, N], F32, name="qb", tag="zc")
    T.matmul(qb, mmdt(C["ones16b_stat"]), mmdt(qfm), start=True, stop=True)
    hq = nodep.tile([128, N + 1, 1], F32, name=f"hq_{rep}", tag="hq")
    V.tensor_tensor(out=hq[:, 0:N, 0], in0=hp, in1=qb, op=Alu.add)
    V.memset(hq[:, N:N + 1, 0], -BIGNEG)
    dbg("hq", hq[:, 0:N, 0])
    if STOP_STAGE == "hq":
        return stop_dma(hq[0:1, 0:1, 0])

    # pooling mask-broadcast prep only needs mfm -> fill the L2-start bubble
    mbhs = []
    for sl, statname in ((0, "maskbc_statA"), (1, "maskbc_statB")):
        mb2 = psum.tile([128, N], F32, name=f"mbp{sl}", tag="zc")
        T.matmul(mb2, mmdt(C[statname]), mmdt(mfm), start=True, stop=True)
        mbh = nodep.tile([128, N], F16, name=f"mbh{sl}_{rep}", tag=f"mbh{sl}")
        S.activation(mbh, mb2, Act.Copy, bias=0.0, scale=1.0)
        mbhs.append(mbh)

    # ================= Layer 2 =================
    D0b, D1b = edge_layer(1, hq, C["t2vec"], "L2")
    if STOP_STAGE == "l2edge":
        return stop_dma(D0b[0:1, 0:1])
    aggr2 = aggr_from_D(D0b, D1b, 1)
    dbg("aggr2", aggr2)
    u2 = nodep.tile([128, N], F32, name=f"u2_{rep}", tag="u2")
    V.scalar_tensor_tensor(out=u2, in0=aggr2, scalar=EPS, in1=hp,
                           op0=Alu.add, op1=Alu.add)
    y2q2 = mlp(u2, C["w2a_stat"], C["b2a_vec"], C["g2_vec"], C["be2_vec"],
               C["w2bh_stat"], 1)
    h2 = [nodep.tile([128, N], F16, name=f"h2{sl}_{rep}", tag=f"h2{sl}")
          for sl in range(2)]
    for sl in range(2):
        S.activation(h2[sl], y2q2[sl], Act.Relu, bias=C["b2b_vec"], scale=1.0)
    dbg("h2a", h2[0]); dbg("h2b", h2[1])
    if STOP_STAGE == "mlp2":
        return stop_dma(h2[0][0:1, 0:1])

    # ================= pooling + head =================
    pooled = []
    for sl in range(2):
        pl = nodep.tile([128, 1], F32, name=f"pl{sl}_{rep}", tag=f"pl{sl}")
        scratch = work.tile([128, N], F16, name="plscratch", tag="plscratch", bufs=1)
        V.scalar_tensor_tensor(out=scratch, in0=h2[sl], scalar=1.0,
                               in1=mbhs[sl], op0=Alu.mult, op1=Alu.mult,
                               accum_out=pl)
        pooled.append(pl)
    P8 = psum1.tile([32, G], F32, name="P8", tag="small")
    for g in range(G):
        sl, gg = g // 4, g % 4
        T.matmul(P8[:, g:g + 1],
                 mmdt(C["selk_stat"][:, 32 * gg:32 * gg + 32]),
                 mmdt(pooled[sl]), start=True, stop=True,
                 skip_group_check=True)
    p8s = nodep.tile([32, G], F32, name=f"p8s_{rep}", tag="p8s")
    S.activation(p8s, P8, Act.Copy, bias=0.0, scale=1.0)
    dbg("pooled", p8s)
    a1p = psum.tile([128, G], F32, name="a1p", tag="zc")
    T.matmul(a1p, mmdt(C["wa_stat"]), mmdt(p8s), start=True, stop=True)
    a1 = nodep.tile([128, G], F32, name=f"a1_{rep}", tag="a1")
    S.activation(a1, a1p, Act.Relu, bias=C["ba_vec"], scale=1.0)
    op = psum1.tile([4, G], F32, name="op", tag="small")
    T.matmul(op, mmdt(C["wo_stat"]), mmdt(a1), start=True, stop=True)
    ot = nodep.tile([4, G], F32, name=f"ot_{rep}", tag="ot")
    S.activation(ot, op, Act.Tanh, bias=C["bo2_vec"], scale=1.0)
    nc.sync.dma_start(out=out_ext, in_=ot)


# ----------------------------------------------------------------------------
# Self-contained entry point: kernel(**inputs) -> [64, 4] float32
# ----------------------------------------------------------------------------
import jax as _jax
from jax.sharding import Mesh as _Mesh, PartitionSpec as _PartitionSpec
from jax.experimental.shard_map import shard_map as _shard_map

_COMPILED = {}


def _build_and_jit():
    """Re-create the jitted executable on every call: re-executing a loaded
    NEFF leaves device state (semaphores) behind and corrupts the second run,
    so each kernel() invocation gets a fresh executable (BIR->NEFF is
    disk-cached, so this costs seconds, not a recompile)."""
    from concourse import bass2jax
    from concourse.bass2jax import _bass_exec_p, partition_id_tensor

    if "nc" in _COMPILED:
        nc = _COMPILED["nc"]
    else:
        nc = build_nc()
        _COMPILED["nc"] = nc
    bass2jax.install_neuronx_cc_hook()
    partition_name = (nc.partition_id_tensor.name
                      if nc.partition_id_tensor else None)
    in_names, out_names, out_avals, zero_outs = [], [], [], []
    for alloc in nc.m.functions[0].allocations:
        if not isinstance(alloc, mybir.MemoryLocationSet):
            continue
        nm = alloc.memorylocations[0].name
        if alloc.kind == "ExternalInput":
            if nm != partition_name:
                in_names.append(nm)
        elif alloc.kind == "ExternalOutput":
            out_names.append(nm)
            out_avals.append(_jax.core.ShapedArray(
                tuple(alloc.tensor_shape), mybir.dt.np(alloc.dtype)))
            zero_outs.append(np.zeros(tuple(alloc.tensor_shape),
                                      mybir.dt.np(alloc.dtype)))
    n_params = len(in_names)
    n_outs = len(out_avals)
    in_names_all = in_names + out_names
    if partition_name is not None:
        in_names_all.append(partition_name)
    donate = tuple(range(n_params, n_params + n_outs))

    def _body(*args):
        operands = list(args)
        if partition_name is not None:
            operands.append(partition_id_tensor())
        return tuple(_bass_exec_p.bind(
            *operands, out_avals=tuple(out_avals),
            in_names=tuple(in_names_all), out_names=tuple(out_names),
            lowering_input_output_aliases=(), sim_require_finite=True,
            sim_require_nnan=True, nc=nc))

    devices = _jax.devices()[:8]
    mesh = _Mesh(np.asarray(devices), ("core",))
    in_specs = (_PartitionSpec("core"),) * (n_params + n_outs)
    out_specs = (_PartitionSpec("core"),) * len(out_names)
    sharded = _jax.jit(
        _shard_map(_body, mesh=mesh, in_specs=in_specs, out_specs=out_specs,
                   check_rep=False),
        donate_argnums=donate, keep_unused=True)
    return (sharded, in_names, out_names, zero_outs)


def kernel(**inputs):
    """Full-input GNN forward on 8 TRN2 NeuronCores; returns [64, 4] f32."""
    sharded, in_names, out_names, zero_outs = _build_and_jit()
    core_maps = prep_inputs(inputs)
    concat_in = [np.concatenate([core_maps[c][nm] for c in range(8)], axis=0)
                 for nm in in_names]
    concat_zero = [np.zeros((8 * z.shape[0], *z.shape[1:]), z.dtype)
                   for z in zero_outs]
    out_arrs = sharded(*concat_in, *concat_zero)
    oi = out_names.index("out")
    full = np.asarray(out_arrs[oi]).reshape(8, 4, G)
    return np.concatenate([full[c].T for c in range(8)], axis=0)



# revision 54
# speedup vs baseline: 1.0133x; 1.0060x over previous
"""GNN (GENConv x2 + TopK pool) Bass/Tile kernel for TRN2, data-parallel over
8 NeuronCores (8 graphs per core).

Per-core layout conventions ("fm" = feature-major packed):
  - Edge tensors:  [128 = 16feat x 8graph, 16384 slots]  (dst-sorted per graph)
    row 16*g + f holds feature f of graph g; free axis = slot (graph-local).
  - Node tensors:  [128 = 16f x 8g, 512 nodes]  (e.g. h, hq, aggr)
    or [128 = 32f x 4g, 512] for 32-dim stages (halves A: graphs 0-3, B: 4-7).
  - Segment sums via chunked f32 prefix scan + boundary gather (ap_gather) in
    2 waves of 8192 slots to bound SBUF.
"""

import numpy as np
from contextlib import ExitStack

import concourse.bass as bass
import concourse.bacc as bacc
import concourse.mybir as mybir
import concourse.tile as tile
from concourse import library_config

F32 = mybir.dt.float32
F32R = mybir.dt.float32r
F16 = mybir.dt.float16
I16 = mybir.dt.int16
I8 = mybir.dt.int8

G = 8          # graphs per core
N = 512        # nodes per graph
EG = 16384     # edges per graph
EF = 16        # edge/node feature dim after encode
XF = 64        # input node feature dim
K = 256        # topk keep
SLOTS = 18432  # padded slots per graph (each node run padded to mult of 4)
Q4 = SLOTS // 4
HQ = Q4 // 2   # boundary-gather table split point
CHUNK = 1024   # slots per elementwise chunk
NCHUNK = SLOTS // CHUNK
PCH = 512      # slots per psum chunk (one bank)
NEND = 576     # padded end-list length (513 used)
EPS = 1e-7
BIGNEG = 2.0e9

USE_F32R = False   # set True once f32r numerics verified on HW
STOP_STAGE = None  # for HW bisects: "l1edge", "aggr1", "mlp1", "rank", "hq", "l2edge", "aggr2", "mlp2"


def mmdt(ap):
    return ap.bitcast(F32R) if USE_F32R else ap


# ----------------------------------------------------------------------------
# Host-side preprocessing: full inputs -> per-core named arrays
# ----------------------------------------------------------------------------

def prep_inputs(inputs: dict) -> list[dict]:
    x = np.asarray(inputs["x"], np.float32)            # [B*N, 64]
    ei = np.asarray(inputs["edge_index"])              # [2, E] int64
    ea = np.asarray(inputs["edge_attr"], np.float32)   # [E, 16]
    B = 64
    assert x.shape == (B * N, XF)
    assert ea.shape == (B * EG, EF)

    src_g = (ei[0] % N).astype(np.int64)
    dst_g = (ei[1] % N).astype(np.int64)
    graph_of_edge = (ei[0] // N).astype(np.int64)
    assert np.array_equal(graph_of_edge, np.repeat(np.arange(B), EG)), \
        "edge blocks not per-graph; prep assumes reference setup_inputs layout"
    assert np.array_equal(ei[0] // N, ei[1] // N)

    def lin(name):
        return np.asarray(inputs[name], np.float32)

    W_ne, b_ne = lin("W_ne"), lin("b_ne")
    W_ee, b_ee = lin("W_ee"), lin("b_ee")
    W1a, b1a, g1, be1 = lin("W1a"), lin("b1a"), lin("g1"), lin("be1")
    W1b, b1b = lin("W1b"), lin("b1b")
    W2a, b2a, g2, be2 = lin("W2a"), lin("b2a"), lin("g2"), lin("be2")
    W2b, b2b = lin("W2b"), lin("b2b")
    Wa, ba, Wo, bo = lin("Wa"), lin("ba"), lin("Wo"), lin("bo")
    w_pool = lin("w_pool")
    wp = w_pool / np.linalg.norm(w_pool)
    t1 = np.float32(inputs["t1"])
    t2 = np.float32(inputs["t2"])

    cst = {}
    wne = np.zeros((128, 4 * 128), np.float32)
    for p in range(4):
        for a in range(2):
            gg = 2 * p + a
            wne[64 * a:64 * a + XF, 128 * p + 16 * gg:128 * p + 16 * gg + EF] = W_ne
    cst["wne_stat"] = wne
    cst["bne_vec"] = np.tile(b_ne, G)[:, None].astype(np.float32)
    wee = np.zeros((128, 128), np.float32)
    for g in range(G):
        wee[16 * g:16 * g + EF, 16 * g:16 * g + EF] = W_ee
    cst["weeh_stat"] = wee.astype(np.float16)
    cst["bee_vec"] = np.tile(b_ee, G)[:, None].astype(np.float32)
    cst["ident"] = np.eye(128, dtype=np.float32)
    cst["identT"] = np.eye(128, dtype=np.float32)
    cst["identh"] = np.eye(128, dtype=np.float16)
    cst["t1vec"] = np.full((128, 1), t1, np.float32)
    cst["t2vec"] = np.full((128, 1), t2, np.float32)
    w1a = np.zeros((64, 128), np.float32)
    for gg in range(4):
        w1a[16 * gg:16 * gg + 16, 32 * gg:32 * gg + 32] = W1a
    cst["w1a_stat"] = np.vstack([w1a, w1a])
    cst["b1a_vec"] = np.tile(b1a, 4)[:, None].astype(np.float32)
    ones32 = np.zeros((128, 4), np.float32)
    for gg in range(4):
        ones32[32 * gg:32 * gg + 32, gg] = 1.0 / 32.0
    cst["ones32_stat"] = ones32
    onesb32 = np.zeros((4, 128), np.float32)
    for gg in range(4):
        onesb32[gg, 32 * gg:32 * gg + 32] = 1.0
    cst["onesb32_stat"] = onesb32
    cst["g1_vec"] = np.tile(g1, 4)[:, None].astype(np.float32)
    cst["be1_vec"] = np.tile(be1, 4)[:, None].astype(np.float32)
    w1b = np.zeros((64, 32), np.float32)
    for gg in range(2):
        w1b[32 * gg:32 * gg + 32, 16 * gg:16 * gg + 16] = W1b
    cst["w1b_stat"] = np.vstack([w1b, w1b])
    cst["b1b_vec"] = np.tile(b1b, G)[:, None].astype(np.float32)
    wpool = np.zeros((128, 8), np.float32)
    for g in range(G):
        wpool[16 * g:16 * g + EF, g] = wp
    cst["wpool_stat"] = wpool
    ones16b = np.zeros((8, 128), np.float32)
    for g in range(G):
        ones16b[g, 16 * g:16 * g + EF] = 1.0
    cst["ones16b_stat"] = ones16b
    cst["ones1x128"] = np.ones((1, 128), np.float32)
    cst["ones8x128"] = np.ones((8, 128), np.float32)
    onesel = np.zeros((8, 8 * 128), np.float32)
    for g in range(8):
        onesel[g, 128 * g:128 * (g + 1)] = 1.0
    cst["onesel_stat"] = onesel
    w2a = np.zeros((64, 128), np.float32)
    for gg in range(4):
        w2a[16 * gg:16 * gg + 16, 32 * gg:32 * gg + 32] = W2a
    cst["w2a_stat"] = np.vstack([w2a, w2a])
    cst["b2a_vec"] = np.tile(b2a, 4)[:, None].astype(np.float32)
    cst["g2_vec"] = np.tile(g2, 4)[:, None].astype(np.float32)
    cst["be2_vec"] = np.tile(be2, 4)[:, None].astype(np.float32)
    w2b = np.zeros((64, 64), np.float32)
    for gg in range(2):
        w2b[32 * gg:32 * gg + 32, 32 * gg:32 * gg + 32] = W2b
    cst["w2b_stat"] = np.vstack([w2b, w2b])
    cst["b2b_vec"] = np.tile(b2b, 4)[:, None].astype(np.float32)
    mbA = np.zeros((8, 128), np.float32)
    mbB = np.zeros((8, 128), np.float32)
    for g in range(4):
        mbA[g, 32 * g:32 * g + 32] = 1.0
        mbB[g + 4, 32 * g:32 * g + 32] = 1.0
    cst["maskbc_statA"] = mbA
    cst["maskbc_statB"] = mbB
    selk = np.zeros((128, 4 * 32), np.float32)
    for gg in range(4):
        selk[32 * gg:32 * gg + 32, 32 * gg:32 * gg + 32] = np.eye(32) / K
    cst["selk_stat"] = selk
    cst["wa_stat"] = Wa.astype(np.float32)
    cst["ba_vec"] = ba[:, None].astype(np.float32)
    cst["wo_stat"] = Wo.astype(np.float32)
    cst["bo2_vec"] = bo[:, None].astype(np.float32)
    cst["lneps_vec"] = np.full((4, 1), 1e-5, np.float32)
    cst["ones32h_stat"] = ones32.astype(np.float16)
    cst["w1bh_stat"] = cst["w1b_stat"].astype(np.float16)
    cst["w2bh_stat"] = cst["w2b_stat"].astype(np.float16)
    cst["big8_vec"] = np.full((128, 1), 1e8, np.float32)
    m1a = (w1a @ ones32).astype(np.float32)
    cst["m1a_stat"] = np.vstack([m1a, m1a])                      # [128, 4]
    cst["m1bias"] = (ones32.T @ np.tile(b1a, 4)[:, None]).astype(np.float32)
    m2a = (w2a @ ones32).astype(np.float32)
    cst["m2a_stat"] = np.vstack([m2a, m2a])
    cst["m2bias"] = (ones32.T @ np.tile(b2a, 4)[:, None]).astype(np.float32)

    core_maps = []
    for core in range(8):
        m = dict(cst)
        gsl = slice(core * G, (core + 1) * G)
        xt = np.zeros((128, 4 * 512), np.float32)
        xs = x.reshape(B, N, XF)[gsl]
        for p in range(4):
            for a in range(2):
                xt[64 * a:64 * a + XF, 512 * p:512 * (p + 1)] = xs[2 * p + a].T
        m["xT"] = xt

        attrT = np.zeros((128, SLOTS), np.float16)
        srcidx = np.full((128, SLOTS // 16), 512, np.int16)
        end4a = np.zeros((128, NEND // 16), np.int16)
        end4b = np.zeros((128, NEND // 16), np.int16)
        selm = np.zeros((128, NEND), np.int8)
        npad = np.zeros((128, N), np.float32)
        for gl in range(G):
            gid = core * G + gl
            s_l = src_g[gid * EG:(gid + 1) * EG]
            d_l = dst_g[gid * EG:(gid + 1) * EG]
            order = np.argsort(d_l, kind="stable")
            ds = d_l[order]
            ea_s = ea[gid * EG:(gid + 1) * EG][order]  # [EG, 16] dst-sorted
            ss = s_l[order]
            ends = np.searchsorted(ds, np.arange(N), side="right")
            starts = np.concatenate([[0], ends[:-1]])
            deg = ends - starts
            pdeg = (deg + 3) // 4 * 4
            off4 = np.concatenate([[0], np.cumsum(pdeg)])
            assert off4[-1] <= SLOTS
            # scatter real edges into padded positions
            pos = off4[ds] + (np.arange(EG) - starts[ds])
            at = np.zeros((SLOTS, EF), np.float16)
            at[pos] = ea_s.astype(np.float16)
            sp = np.full(SLOTS, 512, np.int16)
            sp[pos] = ss.astype(np.int16)
            attrT[16 * gl:16 * gl + EF, :] = at.T
            srcidx[16 * gl:16 * gl + 16, :] = sp.reshape(SLOTS // 16, 16).T
            e4 = np.zeros(NEND, np.int64)
            e4[:513] = off4 // 4
            e4[513:] = e4[512]
            ea4 = np.minimum(e4, HQ).astype(np.int16)
            eb4 = np.clip(e4 - HQ, 0, Q4 - HQ).astype(np.int16)
            end4a[16 * gl:16 * gl + 16, :] = ea4.reshape(NEND // 16, 16).T
            end4b[16 * gl:16 * gl + 16, :] = eb4.reshape(NEND // 16, 16).T
            selm[16 * gl:16 * gl + 16, :] = (e4 <= HQ).astype(np.int8)[None, :]
            npad[16 * gl:16 * gl + 16, :] = (pdeg - deg).astype(np.float32)[None, :]
        m["attrT"] = attrT
        m["srcidx"] = srcidx
        m["end4a"] = end4a
        m["end4b"] = end4b
        m["selm"] = selm
        m["npad"] = npad
        blob = np.zeros((128, CBLOB_BYTES), np.uint8)
        for name, shape, dt, off in CONST_SPECS:
            arr = m[name]
            bv = arr.view(np.uint8).reshape(arr.shape[0], -1)
            blob[:arr.shape[0], off:off + bv.shape[1]] = bv
        core_maps.append({"cblob": blob, "attrT": m["attrT"],
                          "ident": m["ident"],
                          "xT": m["xT"], "srcidx": m["srcidx"]})
    return core_maps


_CSPEC_RAW = [
    ("wne_stat", [128, 4 * 128], F32),
    ("bne_vec", [128, 1], F32),
    ("end4a", [128, NEND // 16], I16),
    ("end4b", [128, NEND // 16], I16),
    ("npad", [128, N], F32),
    ("weeh_stat", [128, 128], F16),
    ("identh", [128, 128], F16),
    ("bee_vec", [128, 1], F32),
    ("identT", [128, 128], F32),
    ("t1vec", [128, 1], F32),
    ("t2vec", [128, 1], F32),
    ("big8_vec", [128, 1], F32),
    ("w1a_stat", [128, 128], F32),
    ("b1a_vec", [128, 1], F32),
    ("ones32_stat", [128, 4], F32),
    ("onesb32_stat", [4, 128], F32),
    ("g1_vec", [128, 1], F32),
    ("be1_vec", [128, 1], F32),
    ("w1b_stat", [128, 32], F32),
    ("b1b_vec", [128, 1], F32),
    ("wpool_stat", [128, 8], F32),
    ("ones16b_stat", [8, 128], F32),
    ("ones1x128", [1, 128], F32),
    ("ones8x128", [8, 128], F32),
    ("onesel_stat", [8, 8 * 128], F32),
    ("w2a_stat", [128, 128], F32),
    ("b2a_vec", [128, 1], F32),
    ("g2_vec", [128, 1], F32),
    ("be2_vec", [128, 1], F32),
    ("w2b_stat", [128, 64], F32),
    ("b2b_vec", [128, 1], F32),
    ("maskbc_statA", [8, 128], F32),
    ("maskbc_statB", [8, 128], F32),
    ("selk_stat", [128, 4 * 32], F32),
    ("wa_stat", [32, 128], F32),
    ("ba_vec", [128, 1], F32),
    ("wo_stat", [128, 4], F32),
    ("bo2_vec", [4, 1], F32),
    ("lneps_vec", [4, 1], F32),
    ("ones32h_stat", [128, 4], F16),
    ("w1bh_stat", [128, 32], F16),
    ("w2bh_stat", [128, 64], F16),
    ("selm", [128, NEND], I8),
    ("m1a_stat", [128, 4], F32),
    ("m1bias", [4, 1], F32),
    ("m2a_stat", [128, 4], F32),
    ("m2bias", [4, 1], F32),
]

def _mk_const_specs():
    specs = []
    off = 0
    import numpy as _np
    for nm, shape, dt in _CSPEC_RAW:
        nbytes = int(_np.prod(shape[1:])) * mybir.dt.size(dt) if len(shape) > 1 else mybir.dt.size(dt)
        nbytes = int(_np.prod(shape[1:])) * mybir.dt.size(dt)
        specs.append((nm, shape, dt, off))
        off += (nbytes + 127) // 128 * 128
    return specs, off

CONST_SPECS, CBLOB_BYTES = _mk_const_specs()

INPUT_SPECS = [
    ("cblob", [128, None], mybir.dt.uint8),
    ("attrT", [128, SLOTS], F16),
    ("ident", [128, 128], F32R),
    ("xT", [128, 4 * 512], F32),
    ("srcidx", [128, SLOTS // 16], I16),
]
INPUT_SPECS[0] = ("cblob", [128, CBLOB_BYTES], mybir.dt.uint8)


# ----------------------------------------------------------------------------
# Device graph
# ----------------------------------------------------------------------------

def build_nc(debug_keys=(), n_rep=1):
    nc = bacc.Bacc(None, target_bir_lowering=False, debug=False)
    A = {}
    for name, shape, dt in INPUT_SPECS:
        A[name] = nc.declare_dram_parameter(name, shape, dt, isOutput=False)[:]
    out_ext = nc.declare_dram_parameter("out", [4, G], F32, isOutput=True)[:]
    dbg_ext = {}
    dbg_shapes = {
        "hT": ((128, N), F32), "hsrc0": ((128, CHUNK), F32),
        "m0": ((128, CHUNK), F16), "ev0": ((128, CHUNK), F16),
        "S0w0": ((128, Q4 + 1), F32), "S1w0": ((128, Q4 + 1), F32),
        "G0": ((128, NEND), F32), "G1": ((128, NEND), F32),
        "aggr1": ((128, N), F32), "h1": ((128, N), F32),
        "score": ((8, N), F32), "snode": ((128, 32), F32),
        "rk": ((128, 32), F32), "mask": ((128, 32), F32),
        "hq": ((128, N), F32), "hp": ((128, N), F32),
        "aggr2": ((128, N), F32), "h2a": ((128, N), F16),
        "h2b": ((128, N), F16), "pooled": ((32, 8), F32),
        "u1": ((128, N), F32), "y1sA": ((128, N), F32), "r1A": ((128, N), F32),
    }
    for key in debug_keys:
        shape, dt = dbg_shapes[key]
        dbg_ext[key] = nc.declare_dram_parameter(
            "dbg_" + key, list(shape), dt, isOutput=True)[:]

    with tile.TileContext(nc) as tc, ExitStack() as ctx:
        consts = ctx.enter_context(tc.tile_pool(name="consts", bufs=1))
        nodep = ctx.enter_context(tc.tile_pool(name="nodep", bufs=1))
        scanp = ctx.enter_context(tc.tile_pool(name="scanp", bufs=1))
        work = ctx.enter_context(tc.tile_pool(name="work", bufs=2))
        psum1 = ctx.enter_context(tc.tile_pool(name="psum1", bufs=2, space="PSUM"))
        psume = ctx.enter_context(tc.tile_pool(name="psume", bufs=2, space="PSUM"))
        psumq = ctx.enter_context(tc.tile_pool(name="psumq", bufs=2, space="PSUM"))
        psum = psume
        psumw = psume

        nc.gpsimd.load_library(library_config.ap_gather)

        C = {}
        cb = consts.tile([128, CBLOB_BYTES], mybir.dt.uint8, name="cblob_sb")
        _s2 = CONST_SPECS[2][3]      # wne+bne only: unblocks hT fast
        _split = CONST_SPECS[12][3]  # rest of the edge-phase constants
        nc.sync.dma_start(out=cb[:, :_s2], in_=A["cblob"][:, :_s2])
        for _n, _shape, _dt in (("srcidx", [128, SLOTS // 16], I16),
                                ("ident", [128, 128], F32R)):
            _t = consts.tile(_shape, _dt, name="c_" + _n)
            nc.scalar.dma_start(out=_t, in_=A[_n])
            C[_n] = _t
        nc.scalar.dma_start(out=cb[:, _s2:_split], in_=A["cblob"][:, _s2:_split])
        _rest = CBLOB_BYTES - _split
        _step = (_rest + 3) // 4
        for _k in range(4):
            _a = _split + _k * _step
            _b = min(_split + (_k + 1) * _step, CBLOB_BYTES)
            if _a < _b:
                nc.scalar.dma_start(out=cb[:, _a:_b], in_=A["cblob"][:, _a:_b])
        for name, shape, dt, off in CONST_SPECS:
            nbytes = int(np.prod(shape[1:])) * mybir.dt.size(dt)
            ap = cb[:shape[0], off:off + nbytes].bitcast(dt)
            if len(shape) == 3:
                ap = ap.rearrange("p (a b) -> p a b", b=shape[2])
            C[name] = ap

        dbg_keys_set = set(debug_keys)

        def dbg(key, ap):
            if key in dbg_keys_set:
                nc.sync.dma_start(out=dbg_ext[key], in_=ap)

        for rep in range(n_rep):
            run_once(nc, tc, A, C, out_ext, dbg, nodep, scanp, work,
                     psum, psum1, psume, psumq, psumw, rep)
    nc.compile()
    return nc


def run_once(nc, tc, A, C, out_ext, dbg, nodep, scanp, work, psum, psum1,
             psume, psumq, psumw, rep):
    V = nc.vector
    S = nc.scalar
    T = nc.tensor
    Alu = mybir.AluOpType
    Act = mybir.ActivationFunctionType

    # ================= node encode: hT = x @ W_ne + b_ne =================
    hraw = psum.tile([128, N], F32, name="hraw", tag="zc")
    for half in range(2):
        xts = work.tile([128, 1024], F32, name=f"xts{half}", tag="xts")
        nc.sync.dma_start(out=xts, in_=A["xT"][:, 1024 * half:1024 * (half + 1)])
        for pp in range(2):
            p = 2 * half + pp
            T.matmul(hraw, mmdt(C["wne_stat"][:, 128 * p:128 * (p + 1)]),
                     mmdt(xts[:, 512 * pp:512 * (pp + 1)]),
                     start=(p == 0), stop=(p == 3))
    hT = nodep.tile([128, N + 1, 1], F32, name="hT")
    S.activation(hT[:, 0:N, 0], hraw, Act.Identity, bias=C["bne_vec"], scale=1.0)
    V.memset(hT[:, N:N + 1, 0], -BIGNEG)  # sentinel column for pad slots
    dbg("hT", hT[:, 0:N, 0])
    zeros512 = nodep.tile([128, PCH], F32, name="zeros512")
    V.memset(zeros512, 0.0)

    # ================= shared edge-phase machinery =================
    # Edge slots are dst-sorted with每 node's run padded to a multiple of 4
    # (pad slots gather the sentinel column -> mc=0, ev=1 in L1 / 0 in L2).
    # Per chunk: zc accumulates in PSUM (wee@attr + ident@hsrc), then 4:1
    # block sums via strided identity matmuls -> short scan -> small gather.
    def edge_layer(layer, table, tvec, dbg_pref):
        S4 = [scanp.tile([128, Q4 + 1, 1], F32, name=f"S4{ti}_{layer}_{rep}",
                         tag=f"S4{ti}") for ti in range(2)]
        V.memset(S4[0][:, 0:1, 0], 0.0)
        V.memset(S4[1][:, 0:1, 0], 0.0)

        for cc in range(NCHUNK):
            base = cc * CHUNK
            attrc = work.tile([128, CHUNK], F16, name="attrc", tag="attrc", bufs=3)
            nc.sync.dma_start(out=attrc, in_=A["attrT"][:, base:base + CHUNK])
            hsrc = work.tile([128, CHUNK, 1], F32, name="hsrc", tag="hsrc", bufs=3)
            j0 = base // 16
            nc.gpsimd.ap_gather(
                hsrc, table, C["srcidx"][:, j0:j0 + CHUNK // 16],
                channels=128, num_elems=N + 1, d=1, num_idxs=CHUNK)
            eap = psume.tile([128, CHUNK], F32, name="zc", tag="zc")
            for s in range(CHUNK // PCH):
                T.matmul(eap[:, PCH * s:PCH * (s + 1)], C["weeh_stat"],
                         attrc[:, PCH * s:PCH * (s + 1)],
                         start=True, stop=True)
            zc = work.tile([128, CHUNK], F32, name="zcs", tag="zcs", bufs=3)
            V.tensor_tensor(out=zc, in0=hsrc[:, :, 0], in1=eap, op=Alu.add)
            mc = work.tile([128, CHUNK], F16, name="mc", tag="mc", bufs=3)
            S.activation(mc, zc, Act.Relu, bias=C["bee_vec"], scale=1.0)
            if layer == 0 and base == 0:
                dbg("hsrc0", hsrc[:, :, 0])
            # evc | evmc packed side by side so the 4:1 reduce runs as one
            # 4-matmul group over both arrays.
            evp = work.tile([128, 2 * CHUNK], F16, name="evp", tag="evp",
                            bufs=3)
            evc = evp[:, 0:CHUNK]
            if layer == 0:
                S.activation(evc, mc, Act.Exp, bias=0.0, scale=tvec)
            else:
                sgn = work.tile([128, CHUNK], F16, name="sgn", tag="sgn")
                S.activation(sgn, zc, Act.Sign, bias=C["big8_vec"], scale=1.0)
                ev0 = work.tile([128, CHUNK], F16, name="ev0t", tag="ev0t",
                                bufs=2)
                S.activation(ev0, mc, Act.Exp, bias=0.0, scale=tvec)
                V.scalar_tensor_tensor(out=evc, in0=sgn, scalar=0.0, in1=ev0,
                                       op0=Alu.max, op1=Alu.mult)
            V.tensor_tensor(out=evp[:, CHUNK:2 * CHUNK], in0=evc, in1=mc,
                            op=Alu.mult)
            if layer == 0 and base == 0:
                dbg("m0", mc); dbg("ev0", evc)
            lo = cc * (CHUNK // 4)
            q4 = psumq.tile([128, CHUNK // 2], F32, name="q4", tag="q4")
            a3 = evp.rearrange("p (a b) -> p a b", b=4)
            for j in range(4):
                T.matmul(q4, C["identh"], a3[:, :, j],
                         start=(j == 0), stop=(j == 3))
            for ti in range(2):
                V.tensor_tensor_scan(
                    out=S4[ti][:, lo + 1:lo + 1 + CHUNK // 4, 0],
                    data0=q4[:, ti * (CHUNK // 4):(ti + 1) * (CHUNK // 4)],
                    data1=zeros512[:, 0:CHUNK // 4],
                    initial=S4[ti][:, lo:lo + 1, 0],
                    op0=Alu.add, op1=Alu.add)

        D = []
        GA = []
        for ti in range(2):
            # first-half gather only needs chunks 0..8 -> fires mid-layer
            gwa = nodep.tile([128, NEND, 1], F32,
                             name=f"gwa{ti}_{layer}_{rep}", tag=f"gwa{ti}")
            nc.gpsimd.ap_gather(
                gwa, S4[ti][:, 0:HQ + 1, :], C["end4a"],
                channels=128, num_elems=HQ + 1, d=1, num_idxs=NEND)
            GA.append(gwa)
        for ti in range(2):
            gwb = nodep.tile([128, NEND, 1], F32,
                             name=f"gwb{ti}_{layer}_{rep}", tag=f"gwb{ti}")
            nc.gpsimd.ap_gather(
                gwb, S4[ti][:, HQ:Q4 + 1, :], C["end4b"],
                channels=128, num_elems=Q4 - HQ + 1, d=1, num_idxs=NEND)
            gsel = nodep.tile([128, NEND], F32,
                              name=f"gsel{ti}_{layer}_{rep}", tag=f"gsel{ti}")
            V.select(gsel, C["selm"], GA[ti][:, :, 0], gwb[:, :, 0])
            dbg("G0" if ti == 0 else "G1", gsel)
            d = nodep.tile([128, N], F32, name=f"D{ti}_{layer}_{rep}",
                           tag=f"D{ti}")
            V.tensor_tensor(out=d, in0=gsel[:, 1:N + 1], in1=gsel[:, 0:N],
                            op=Alu.subtract)
            D.append(d)
        if layer == 0:
            d0c = nodep.tile([128, N], F32, name=f"D0c_{layer}_{rep}",
                             tag="D0c")
            V.tensor_tensor(out=d0c, in0=D[0], in1=C["npad"], op=Alu.subtract)
            D[0] = d0c
        return D

    def aggr_from_D(D0, D1, layer):
        dm = nodep.tile([128, N], F32, name=f"dm_{layer}_{rep}", tag="dm")
        V.tensor_scalar(dm, D0, 1e-16, None, Alu.max)
        rec = nodep.tile([128, N], F32, name=f"rec_{layer}_{rep}", tag="rec")
        V.reciprocal(rec, dm)
        ag = nodep.tile([128, N], F32, name=f"ag_{layer}_{rep}", tag="ag")
        V.tensor_tensor(out=ag, in0=D1, in1=rec, op=Alu.mult)
        return ag

    def mlp(uin, wa_stat, ba_vec, gvec, bevec, wb_stat, layer):
        N2 = 2 * N
        y1p = psumw.tile([128, 2, N], F32, name=f"y1p_{layer}", tag="zc")
        for half in range(2):
            T.matmul(y1p[:, half, :], mmdt(wa_stat[64 * half:64 * half + 64, :]),
                     mmdt(uin[64 * half:64 * half + 64, :]),
                     start=True, stop=True)
        y1pf = y1p.rearrange("p a b -> p (a b)")
        y1s = nodep.tile([128, N2], F32, name=f"y1s_{layer}_{rep}", tag="y1s")
        S.activation(y1s, y1pf, Act.Identity, bias=ba_vec, scale=1.0)
        if layer == 0:
            dbg("y1sA", y1s[:, 0:N])
        mstat = C["m1a_stat"] if layer == 0 else C["m2a_stat"]
        mbias = C["m1bias"] if layer == 0 else C["m2bias"]
        mp = psumw.tile([4, 2, N], F32, name=f"mp_{layer}", tag="zc")
        for half in range(2):
            T.matmul(mp[:, half, :], mmdt(mstat[64 * half:64 * half + 64, :]),
                     mmdt(uin[64 * half:64 * half + 64, :]),
                     start=True, stop=True)
        ms = nodep.tile([4, N2], F32, name=f"ms_{layer}_{rep}", tag="st4", bufs=2)
        S.activation(ms, mp.rearrange("p a b -> p (a b)"), Act.Identity,
                     bias=mbias, scale=1.0)
        mb = psumw.tile([128, 2, N], F32, name=f"mb_{layer}", tag="zc")
        for half in range(2):
            T.matmul(mb[:, half, :], mmdt(C["onesb32_stat"]),
                     mmdt(ms[:, half * N:half * N + N]), start=True, stop=True)
        yc = nodep.tile([128, N2], F32, name=f"yc_{layer}_{rep}", tag="yc")
        V.tensor_tensor(out=yc, in0=y1s, in1=mb.rearrange("p a b -> p (a b)"),
                        op=Alu.subtract)
        sq = nodep.tile([128, N2], F16, name=f"sq_{layer}_{rep}", tag="sq")
        S.activation(sq, yc, Act.Square, bias=0.0, scale=1.0)
        vp = psumw.tile([4, 2, N], F32, name=f"vp_{layer}", tag="zc")
        for half in range(2):
            T.matmul(vp[:, half, :], C["ones32h_stat"],
                     sq[:, half * N:half * N + N], start=True, stop=True)
        lnv = nodep.tile([4, N2], F32, name=f"lnv_{layer}_{rep}", tag="st4", bufs=2)
        S.activation(lnv, vp.rearrange("p a b -> p (a b)"), Act.Ln,
                     bias=C["lneps_vec"], scale=1.0)
        rstd = nodep.tile([4, N2], F32, name=f"rstd_{layer}_{rep}", tag="st4", bufs=2)
        S.activation(rstd, lnv, Act.Exp, bias=0.0, scale=-0.5)
        rb = psumw.tile([128, 2, N], F32, name=f"rb_{layer}", tag="zc")
        for half in range(2):
            T.matmul(rb[:, half, :], mmdt(C["onesb32_stat"]),
                     mmdt(rstd[:, half * N:half * N + N]), start=True, stop=True)
        vnorm = nodep.tile([128, N2], F32, name=f"vn_{layer}_{rep}", tag="vn")
        V.tensor_tensor(out=vnorm, in0=yc, in1=rb.rearrange("p a b -> p (a b)"),
                        op=Alu.mult)
        r1 = nodep.tile([128, N2], F16, name=f"r1_{layer}_{rep}", tag="r1")
        S.activation(r1, vnorm, Act.Relu, bias=bevec, scale=gvec)
        if layer == 0:
            dbg("r1A", r1[:, 0:N])
        M = wb_stat.shape[1]
        outs = [psum.tile([2 * M, N], F32, name=f"yqb{b}_{layer}", tag="zc")
                for b in range(2)]
        idx = 0
        for half in range(2):
            for q in range(2):
                bank, slot = divmod(idx, 2)
                T.matmul(outs[bank][slot * M:(slot + 1) * M, :],
                         wb_stat[64 * q:64 * q + 64, :],
                         r1[64 * q:64 * q + 64, half * N:half * N + N],
                         start=True, stop=True, skip_group_check=True)
                idx += 1
        return outs

    # ================= Layer 1 =================
    D0, D1 = edge_layer(0, hT, C["t1vec"], "")
    if STOP_STAGE == "l1edge":
        return stop_dma(D0[0:1, 0:1])
    aggr1 = aggr_from_D(D0, D1, 0)
    dbg("aggr1", aggr1)
    if STOP_STAGE == "aggr1":
        return stop_dma(aggr1[0:1, 0:1])
    u1 = nodep.tile([128, N], F32, name=f"u1_{rep}", tag="u1")
    V.scalar_tensor_tensor(out=u1, in0=aggr1, scalar=EPS, in1=hT[:, 0:N, 0],
                           op0=Alu.add, op1=Alu.add)
    dbg("u1", u1)
    y2q = mlp(u1, C["w1a_stat"], C["b1a_vec"], C["g1_vec"], C["be1_vec"],
              C["w1bh_stat"], 0)
    h1 = nodep.tile([128, N], F32, name=f"h1_{rep}", tag="h1")
    for b in range(2):
        S.activation(h1[64 * b:64 * (b + 1), :], y2q[b], Act.Relu,
                     bias=C["b1b_vec"][64 * b:64 * (b + 1), :], scale=1.0)
    dbg("h1", h1)
    if STOP_STAGE == "mlp1":
        return stop_dma(h1[0:1, 0:1])

    # ================= score / topk mask / gates =================
    scp = psum.tile([8, N], F32, name="scp", tag="zc")
    T.matmul(scp, C["wpool_stat"], h1, start=True, stop=True)
    scs = nodep.tile([8, N], F32, name=f"scs_{rep}", tag="scs")
    S.activation(scs, scp, Act.Copy, bias=0.0, scale=1.0)
    dbg("score", scs)
    snode = nodep.tile([128, 4, 8], F32, name=f"snode_{rep}", tag="snode")
    for t in range(4):
        tp = psum1.tile([128, 8], F32, name="tp", tag="small")
        T.transpose(tp, scs[:, 128 * t:128 * (t + 1)], C["identT"][0:8, 0:8])
        S.activation(snode[:, t, :], tp, Act.Copy, bias=0.0, scale=1.0)
    dbg("snode", snode.rearrange("p a b -> p (a b)"))
    sneg = nodep.tile([128, 4, 8], F32, name=f"sneg_{rep}", tag="sneg")
    V.tensor_scalar(sneg, snode, -1.0, None, Alu.mult)
    rk = nodep.tile([128, 4, 8], F32, name=f"rk_{rep}", tag="rk")
    sgnscratch = work.tile([128, N], F16, name="sgnscratch", tag="plscratch", bufs=1)
    gtscratch = work.tile([128, N], F16, name="gtscratch", tag="gtscratch", bufs=1)
    for g in range(G):
        sb = psum.tile([128, N], F32, name="sb", tag="zc")
        T.matmul(sb, mmdt(C["onesel_stat"][:, 128 * g:128 * (g + 1)]),
                 mmdt(scs), start=True, stop=True)
        # t 0/1 on Act (rk = sum of signs, keep iff rk <= -1);
        # t 2/3 on DVE (rk = count greater, keep iff rk <= 255.5)
        for t in range(2):
            S.activation(sgnscratch, sb, Act.Sign,
                         bias=sneg[:, t, g:g + 1], scale=1.0,
                         accum_out=rk[:, t, g:g + 1])
        for t in range(2, 4):
            V.tensor_scalar(gtscratch, sb, snode[:, t, g:g + 1], 0.0,
                            Alu.is_gt, Alu.add,
                            accum_out=rk[:, t, g:g + 1])
    dbg("rk", rk.rearrange("p a b -> p (a b)"))
    tst3 = nodep.tile([128, 4, 96], F32, name=f"tst3_{rep}", tag="tst3")
    mask01 = tst3[:, :, 64:72]
    V.tensor_scalar(mask01[:, 0:2, :], rk[:, 0:2, :], -1.0, None, Alu.is_le)
    V.tensor_scalar(mask01[:, 2:4, :], rk[:, 2:4, :], 255.5, None, Alu.is_le)
    th = nodep.tile([128, 4, 8], F32, name=f"th_{rep}", tag="th")
    S.activation(th, snode, Act.Tanh, bias=0.0, scale=1.0)
    V.tensor_tensor(out=tst3[:, :, 0:8], in0=th, in1=mask01, op=Alu.mult)
    V.tensor_scalar(tst3[:, :, 32:40], mask01, -1.0, BIGNEG, Alu.add, Alu.mult)
    gfm = nodep.tile([8, N], F32, name=f"gfm_{rep}", tag="gfm")
    qfm = nodep.tile([8, N], F32, name=f"qfm_{rep}", tag="qfm")
    mfm = nodep.tile([8, N], F32, name=f"mfm_{rep}", tag="mfm")
    for t in range(4):
        tq = psum1.tile([96, 128], F32, name="tq", tag="small")
        T.transpose(tq, tst3[:, t, :], C["identT"])
        S.activation(gfm[:, 128 * t:128 * (t + 1)], tq[0:8, :], Act.Copy,
                     bias=0.0, scale=1.0)
        S.activation(qfm[:, 128 * t:128 * (t + 1)], tq[32:40, :], Act.Copy,
                     bias=0.0, scale=1.0)
        S.activation(mfm[:, 128 * t:128 * (t + 1)], tq[64:72, :], Act.Copy,
                     bias=0.0, scale=1.0)
    gb = psum.tile([128, N], F32, name="gb", tag="zc")
    T.matmul(gb, mmdt(C["ones16b_stat"]), mmdt(gfm), start=True, stop=True)
    hp = nodep.tile([128, N], F32, name=f"hp_{rep}", tag="hp")
    V.tensor_tensor(out=hp, in0=h1, in1=gb, op=Alu.mult)
    dbg("hp", hp)
    qb = psum.tile([128, N], F32, name="qb", tag="zc")
    T.matmul(qb, mmdt(C["ones16b_stat"]), mmdt(qfm), start=True, stop=True)
    hq = nodep.tile([128, N + 1, 1], F32, name=f"hq_{rep}", tag="hq")
    V.tensor_tensor(out=hq[:, 0:N, 0], in0=hp, in1=qb, op=Alu.add)
    V.memset(hq[:, N:N + 1, 0], -BIGNEG)
    dbg("hq", hq[:, 0:N, 0])
    if STOP_STAGE == "hq":
        return stop_dma(hq[0:1, 0:1, 0])

    # pooling mask-broadcast prep only needs mfm -> fill the L2-start bubble
    mbhs = []
    for sl, statname in ((0, "maskbc_statA"), (1, "maskbc_statB")):
        mb2 = psum.tile([128, N], F32, name=f"mbp{sl}", tag="zc")
        T.matmul(mb2, mmdt(C[statname]), mmdt(mfm), start=True, stop=True)
        mbh = nodep.tile([128, N], F16, name=f"mbh{sl}_{rep}", tag=f"mbh{sl}")
        S.activation(mbh, mb2, Act.Copy, bias=0.0, scale=1.0)
        mbhs.append(mbh)

    # ================= Layer 2 =================
    D0b, D1b = edge_layer(1, hq, C["t2vec"], "L2")
    if STOP_STAGE == "l2edge":
        return stop_dma(D0b[0:1, 0:1])
    aggr2 = aggr_from_D(D0b, D1b, 1)
    dbg("aggr2", aggr2)
    u2 = nodep.tile([128, N], F32, name=f"u2_{rep}", tag="u2")
    V.scalar_tensor_tensor(out=u2, in0=aggr2, scalar=EPS, in1=hp,
                           op0=Alu.add, op1=Alu.add)
    y2q2 = mlp(u2, C["w2a_stat"], C["b2a_vec"], C["g2_vec"], C["be2_vec"],
               C["w2bh_stat"], 1)
    h2 = [nodep.tile([128, N], F16, name=f"h2{sl}_{rep}", tag=f"h2{sl}")
          for sl in range(2)]
    for sl in range(2):
        S.activation(h2[sl], y2q2[sl], Act.Relu, bias=C["b2b_vec"], scale=1.0)
    dbg("h2a", h2[0]); dbg("h2b", h2[1])
    if STOP_STAGE == "mlp2":
        return stop_dma(h2[0][0:1, 0:1])

    # ================= pooling + head =================
    pooled = []
    for sl in range(2):
        pl = nodep.tile([128, 1], F32, name=f"pl{sl}_{rep}", tag=f"pl{sl}")
        scratch = work.tile([128, N], F16, name="plscratch", tag="plscratch", bufs=1)
        V.scalar_tensor_tensor(out=scratch, in0=h2[sl], scalar=1.0,
                               in1=mbhs[sl], op0=Alu.mult, op1=Alu.mult,
                               accum_out=pl)
        pooled.append(pl)
    P8 = psum1.tile([32, G], F32, name="P8", tag="small")
    for g in range(G):
        sl, gg = g // 4, g % 4
        T.matmul(P8[:, g:g + 1],
                 mmdt(C["selk_stat"][:, 32 * gg:32 * gg + 32]),
                 mmdt(pooled[sl]), start=True, stop=True,
                 skip_group_check=True)
    p8s = nodep.tile([32, G], F32, name=f"p8s_{rep}", tag="p8s")
    S.activation(p8s, P8, Act.Copy, bias=0.0, scale=1.0)
    dbg("pooled", p8s)
    a1p = psum.tile([128, G], F32, name="a1p", tag="zc")
    T.matmul(a1p, mmdt(C["wa_stat"]), mmdt(p8s), start=True, stop=True)
    a1 = nodep.tile([128, G], F32, name=f"a1_{rep}", tag="a1")
    S.activation(a1, a1p, Act.Relu, bias=C["ba_vec"], scale=1.0)
    op = psum1.tile([4, G], F32, name="op", tag="small")
    T.matmul(op, mmdt(C["wo_stat"]), mmdt(a1), start=True, stop=True)
    ot = nodep.tile([4, G], F32, name=f"ot_{rep}", tag="ot")
    S.activation(ot, op, Act.Tanh, bias=C["bo2_vec"], scale=1.0)
    nc.sync.dma_start(out=out_ext, in_=ot)


# ----------------------------------------------------------------------------
# Self-contained entry point: kernel(**inputs) -> [64, 4] float32
# ----------------------------------------------------------------------------
import jax as _jax
from jax.sharding import Mesh as _Mesh, PartitionSpec as _PartitionSpec
from jax.experimental.shard_map import shard_map as _shard_map

_COMPILED = {}


def _build_and_jit():
    """Re-create the jitted executable on every call: re-executing a loaded
    NEFF leaves device state (semaphores) behind and corrupts the second run,
    so each kernel() invocation gets a fresh executable (BIR->NEFF is
    disk-cached, so this costs seconds, not a recompile)."""
    from concourse import bass2jax
    from concourse.bass2jax import _bass_exec_p, partition_id_tensor

    if "nc" in _COMPILED:
        nc = _COMPILED["nc"]
    else:
        nc = build_nc()
        _COMPILED["nc"] = nc
    bass2jax.install_neuronx_cc_hook()
    partition_name = (nc.partition_id_tensor.name
                      if nc.partition_id_tensor else None)
    in_names, out_names, out_avals, zero_outs = [], [], [], []
    for alloc in nc.m.functions[0].allocations:
        if not isinstance(alloc, mybir.MemoryLocationSet):
            continue
        nm = alloc.memorylocations[0].name
        if alloc.kind == "ExternalInput":
            if nm != partition_name:
                in_names.append(nm)
        elif alloc.kind == "ExternalOutput":
            out_names.append(nm)
            out_avals.append(_jax.core.ShapedArray(
                tuple(alloc.tensor_shape), mybir.dt.np(alloc.dtype)))
            zero_outs.append(np.zeros(tuple(alloc.tensor_shape),
                                      mybir.dt.np(alloc.dtype)))
    n_params = len(in_names)
    n_outs = len(out_avals)
    in_names_all = in_names + out_names
    if partition_name is not None:
        in_names_all.append(partition_name)
    donate = tuple(range(n_params, n_params + n_outs))

    def _body(*args):
        operands = list(args)
        if partition_name is not None:
            operands.append(partition_id_tensor())
        return tuple(_bass_exec_p.bind(
            *operands, out_avals=tuple(out_avals),
            in_names=tuple(in_names_all), out_names=tuple(out_names),
            lowering_input_output_aliases=(), sim_require_finite=True,
            sim_require_nnan=True, nc=nc))

    devices = _jax.devices()[:8]
    mesh = _Mesh(np.asarray(devices), ("core",))
    in_specs = (_PartitionSpec("core"),) * (n_params + n_outs)
    out_specs = (_PartitionSpec("core"),) * len(out_names)
    sharded = _jax.jit(
        _shard_map(_body, mesh=mesh, in_specs=in_specs, out_specs=out_specs,
                   check_rep=False),
        donate_argnums=donate, keep_unused=True)
    return (sharded, in_names, out_names, zero_outs)


def kernel(**inputs):
    """Full-input GNN forward on 8 TRN2 NeuronCores; returns [64, 4] f32."""
    sharded, in_names, out_names, zero_outs = _build_and_jit()
    core_maps = prep_inputs(inputs)
    concat_in = [np.concatenate([core_maps[c][nm] for c in range(8)], axis=0)
                 for nm in in_names]
    concat_zero = [np.zeros((8 * z.shape[0], *z.shape[1:]), z.dtype)
                   for z in zero_outs]
    out_arrs = sharded(*concat_in, *concat_zero)
    oi = out_names.index("out")
    full = np.asarray(out_arrs[oi]).reshape(8, 4, G)
    return np.concatenate([full[c].T for c in range(8)], axis=0)

